# revision 11
# baseline (speedup 1.0000x reference)
"""Trainium2 Bass kernel for nn_CachedConditionNumberLoss.

Computes loss = log(lambda_max) - log(lambda_min) of M = L A L^T where
A = G G^T/n + I  (G = A_factor, n = 2048) and L = I + scatter(pred*scale).

Strategy (8-core SPMD, column-panel sharded):
  - core i owns the column panel X[:, i*PW:(i+1)*PW] of every 2048x2048
    matrix involved; all cross-core exchange is AllGather of panels.
  - device computes A = G G^T/n + I, W = A L^T, M = L W (three panel
    matmul passes), then extremal eigenvalues of M via two repeated-
    squaring chains with trace-ratio estimators:
      chain 1 on M          -> lambda_max
      chain 2 on mu*I - M   -> mu - lambda_min   (mu = 1.001*lambda_max)
    Each squaring step: AllGather panels of X_k (with the Frobenius-norm
    partial embedded in a tail row), then each core computes
    X_{k+1}[:, panel] = (X_k^T X_k)[:, panel] / t_k^2 with PE matmuls
    (lhsT tiles are read straight from the gathered copy; symmetry of X_k
    makes transposes unnecessary).  Trace bookkeeping on device:
      t_{k+1} = ||X_k||_F^2 / t_k^2,  tau_k = ln t_k,
      s_{k+1} = 2 (s_k + tau_k),
      ln(lam_hat) = (s_K + tau_K + tau_{K+1}) / 2^K.
  - host only transposes/slices inputs (incl. assembling L^T from the
    scatter triplets) and reads back the scalar.
"""

import numpy as np

import concourse.tile as tile
from concourse import bacc, mybir
from concourse.bass_utils import run_bass_kernel_spmd

F32 = mybir.dt.float32
ACT = mybir.ActivationFunctionType
ALU = mybir.AluOpType
P = 128
N_CORES = 8

# squaring-chain lengths (validated in fp32 simulation: loss relerr ~2e-7)
K1 = 11
K2 = 14
MU_FACTOR = 1.001


def _build_nc(n=2048, k1=K1, k2=K2, mm_dt=F32, debug_stage=None):
    ch = n // P           # 128-row chunks per matrix (16)
    pw = n // N_CORES     # panel width per core (256)
    cw = ch * pw          # panel free size in SBUF layout (4096)
    agr = P + 1           # rows per rank in AG buffers (tail row at P)
    cpp = pw // P         # column chunks per panel (2)

    nc = bacc.Bacc(None, target_bir_lowering=False)

    # panelized full matrices (same layout as AG outputs, tail rows unused)
    gt_pan_full = nc.dram_tensor("gt_pan_full", [N_CORES * agr, cw], F32,
                                 kind="ExternalInput")
    lt_pan_full = nc.dram_tensor("lt_pan_full", [N_CORES * agr, cw], F32,
                                 kind="ExternalInput")
    git_pan = nc.dram_tensor("git_pan", [P, cw], F32, kind="ExternalInput")
    lti_pan = nc.dram_tensor("lti_pan", [P, cw], F32, kind="ExternalInput")
    ei_pan = nc.dram_tensor("ei_pan", [P, cw], F32, kind="ExternalInput")

    loss_out = nc.dram_tensor("loss", [1, 1], F32, kind="ExternalOutput")
    dbg_out = nc.dram_tensor("dbg", [1, 8], F32, kind="ExternalOutput")

    m_store = nc.dram_tensor("m_store", [P, cw], F32, kind="Internal")
    pan_out = (nc.dram_tensor("pan_out", [P, cw], F32, kind="ExternalOutput")
               if debug_stage in ("A", "W", "M") else None)

    with tile.TileContext(nc) as tc:
        with (
            tc.tile_pool(name="xf", bufs=7) as xf_pool,
            tc.tile_pool(name="pan", bufs=3) as pan_pool,
            tc.tile_pool(name="eip", bufs=1) as ei_pool,
            tc.tile_pool(name="small", bufs=6) as sm_pool,
            tc.tile_pool(name="state", bufs=1) as st_pool,
            tc.tile_pool(name="psum", bufs=6, space="PSUM") as ps_pool,
            tc.tile_pool(name="psr", bufs=2, space="PSUM") as psr_pool,
            tc.tile_pool(name="dram", bufs=2, space="DRAM") as dram_pool,
        ):
            _trace_program(
                nc, n, k1, k2, mm_dt, debug_stage,
                ch, pw, cw, agr, cpp,
                gt_pan_full, lt_pan_full, git_pan, lti_pan, ei_pan,
                loss_out, dbg_out, m_store, pan_out,
                xf_pool, pan_pool, ei_pool, sm_pool, st_pool,
                ps_pool, psr_pool, dram_pool,
            )

    nc.compile()
    return nc


def _trace_program(nc, n, k1, k2, mm_dt, debug_stage,
                   ch, pw, cw, agr, cpp,
                   gt_pan_full, lt_pan_full, git_pan, lti_pan, ei_pan,
                   loss_out, dbg_out, m_store, pan_out,
                   xf_pool, pan_pool, ei_pool, sm_pool, st_pool,
                   ps_pool, psr_pool, dram_pool):
    ones = st_pool.tile([P, P], F32)
    nc.vector.memset(ones[:], 1.0)

    ei = ei_pool.tile([P, cw], F32, tag="ei")
    nc.sync.dma_start(ei[:], ei_pan[:])

    # ---------- helpers ----------
    def part_reduce(vec_ap, width=1):
        """[p, width] -> [P, width] replicated column sums."""
        red = psr_pool.tile([P, 2], F32, space="PSUM", tag="red")
        p_sz = vec_ap.shape[0]
        nc.tensor.matmul(red[:, 0:width], lhsT=ones[:p_sz, :],
                         rhs=vec_ap, start=True, stop=True)
        out = sm_pool.tile([P, width], F32, tag="pred")
        nc.vector.tensor_copy(out[:], red[:, 0:width])
        return out

    def fnorm_partial(pan_tile):
        """sum of squares of a [P, cw] panel -> [P,1] replicated."""
        acc = sm_pool.tile([P, ch], F32, tag="facc")
        for c in range(ch):
            tmp = sm_pool.tile([P, pw], F32, tag="sqtmp")
            nc.scalar.activation(tmp[:], pan_tile[:, c * pw:(c + 1) * pw],
                                 ACT.Square, accum_out=acc[:, c:c + 1])
        accs = sm_pool.tile([P, 1], F32, tag="faccs")
        nc.vector.reduce_sum(accs[:], acc[:], axis=mybir.AxisListType.X)
        return part_reduce(accs[:])

    def diag_partial(pan_tile):
        """sum of (panel .* ei) -> [P,1] replicated local trace partial."""
        acc = sm_pool.tile([P, ch], F32, tag="facc")
        for c in range(ch):
            sl = slice(c * pw, (c + 1) * pw)
            tmp = sm_pool.tile([P, pw], F32, tag="sqtmp")
            nc.vector.tensor_tensor(out=tmp[:], in0=pan_tile[:, sl],
                                    in1=ei[:, sl], op=ALU.mult)
            nc.vector.reduce_sum(acc[:, c:c + 1], tmp[:],
                                 axis=mybir.AxisListType.X)
        accs = sm_pool.tile([P, 1], F32, tag="faccs")
        nc.vector.reduce_sum(accs[:], acc[:], axis=mybir.AxisListType.X)
        return part_reduce(accs[:])

    def mm_pass(src_dram, rhs_tile, evict_fn):
        """out[:, panel] = X^T @ rhs_panel, X stored panelized in src_dram."""
        tiles = []
        for r in range(N_CORES):
            t = xf_pool.tile([P, cw], mm_dt, tag="xf")
            nc.sync.dma_start(t[:], src_dram[r * agr:r * agr + P, :])
            tiles.append(t)
        for m in range(ch):
            acc = ps_pool.tile([P, pw], F32, space="PSUM", tag="mm")
            t = tiles[m // cpp]
            base = (m % cpp) * P
            for k in range(ch):
                nc.tensor.matmul(
                    acc[:],
                    lhsT=t[:, k * pw + base:k * pw + base + P],
                    rhs=rhs_tile[:, k * pw:(k + 1) * pw],
                    start=(k == 0), stop=(k == ch - 1),
                )
            evict_fn(m, acc[:])

    def do_allgather(pan_tile, tail_tile):
        """AG panels + 2-value tail; returns (ag_out, totals[P,2])."""
        ag_in = dram_pool.tile([agr, cw], F32, tag="agin")
        ag_out = dram_pool.tile([N_CORES * agr, cw], F32, tag="agout",
                                addr_space="Shared")
        nc.sync.dma_start(ag_in[0:P, :], pan_tile[:])
        nc.sync.dma_start(ag_in[P:P + 1, 0:2], tail_tile[0:1, 0:2])
        nc.gpsimd.collective_compute(
            "AllGather", ALU.bypass,
            ins=[ag_in[:]], outs=[ag_out[:]],
            replica_groups=[list(range(N_CORES))],
        )
        tails8 = sm_pool.tile([N_CORES, 2], F32, tag="tails8")
        nc.sync.dma_start(
            tails8[:],
            ag_out.rearrange("(r p) c -> r p c", p=agr)[:, P:P + 1, 0:2])
        totals = part_reduce(tails8[:], width=2)
        return ag_out, totals

    def tiny_allgather(tail_tile):
        agt_in = dram_pool.tile([1, 16], F32, tag="agtin")
        agt_out = dram_pool.tile([N_CORES, 16], F32, tag="agtout",
                                 addr_space="Shared")
        pad = sm_pool.tile([1, 16], F32, tag="tailpad")
        nc.vector.memset(pad[:], 0.0)
        nc.vector.tensor_copy(pad[:, 0:2], tail_tile[0:1, 0:2])
        nc.sync.dma_start(agt_in[:], pad[:])
        nc.gpsimd.collective_compute(
            "AllGather", ALU.bypass,
            ins=[agt_in[:]], outs=[agt_out[:]],
            replica_groups=[list(range(N_CORES))],
        )
        t8 = sm_pool.tile([N_CORES, 2], F32, tag="tails8")
        nc.sync.dma_start(t8[:], agt_out[:, 0:2])
        return part_reduce(t8[:], width=2)

    def make_tail(f_rep, aux_rep=None):
        t = sm_pool.tile([1, 2], F32, tag="tail")
        nc.vector.tensor_copy(t[:, 0:1], f_rep[0:1, :])
        if aux_rep is not None:
            nc.vector.tensor_copy(t[:, 1:2], aux_rep[0:1, :])
        else:
            nc.vector.memset(t[:, 1:2], 0.0)
        return t

    def _dbg_finish(tile_):
        nc.sync.dma_start(pan_out[:], tile_[:])
        z = sm_pool.tile([1, 2], F32, tag="tail")
        nc.vector.memset(z[:], 0.0)
        nc.sync.dma_start(loss_out[:], z[0:1, 0:1])
        d = sm_pool.tile([1, 8], F32, tag="dbgv")
        nc.vector.memset(d[:], 0.0)
        nc.sync.dma_start(dbg_out[:], d[:])

    # ---------- formation: A = G G^T / n + I ----------
    gpan = pan_pool.tile([P, cw], mm_dt, tag="pan")
    nc.sync.dma_start(gpan[:], git_pan[:])
    apan = pan_pool.tile([P, cw], mm_dt, tag="pan")

    def evict_a(m, psum_ap):
        sl = slice(m * pw, (m + 1) * pw)
        nc.scalar.activation(apan[:, sl], psum_ap, ACT.Copy, scale=1.0 / n)
        nc.vector.tensor_add(apan[:, sl], apan[:, sl], ei[:, sl])

    mm_pass(gt_pan_full[:], gpan, evict_a)

    if debug_stage == "A":
        _dbg_finish(apan)
        return

    # ---------- AG(A); W = A L^T ----------
    zt = sm_pool.tile([1, 2], F32, tag="tail")
    nc.vector.memset(zt[:], 0.0)
    agA, _ = do_allgather(apan, zt)

    ltpan = pan_pool.tile([P, cw], mm_dt, tag="pan")
    nc.sync.dma_start(ltpan[:], lti_pan[:])
    wpan = pan_pool.tile([P, cw], mm_dt, tag="pan")

    def evict_plain(dst):
        def fn(m, psum_ap):
            sl = slice(m * pw, (m + 1) * pw)
            nc.scalar.activation(dst[:, sl], psum_ap, ACT.Copy)
        return fn

    mm_pass(agA[:], ltpan, evict_plain(wpan))

    if debug_stage == "W":
        _dbg_finish(wpan)
        return

    # ---------- M = L W ----------
    mpan = pan_pool.tile([P, cw], mm_dt, tag="pan")
    mm_pass(lt_pan_full[:], wpan, evict_plain(mpan))
    nc.sync.dma_start(m_store[:], mpan[:])

    if debug_stage == "M":
        _dbg_finish(mpan)
        return

    # persistent chain state
    t_cur = st_pool.tile([P, 1], F32)
    s_acc = st_pool.tile([P, 1], F32)
    ln_lam1 = st_pool.tile([P, 1], F32)
    mu = st_pool.tile([P, 1], F32)
    trMg = st_pool.tile([P, 1], F32)   # global trace of M

    def chain(x0_tile, K, init_t_fn, aux0=None):
        """Squaring chain; returns ln(lam_hat) as a [P,1] tile."""
        nc.vector.memset(s_acc[:], 0.0)
        xpan = x0_tile
        f_rep = fnorm_partial(xpan)
        for k in range(K + 1):
            tail = make_tail(f_rep, aux0 if k == 0 else None)
            if k < K:
                ag_out, totals = do_allgather(xpan, tail)
            else:
                totals = tiny_allgather(tail)
            if k == 0:
                init_t_fn(totals)
            tau = sm_pool.tile([P, 1], F32, tag="tau")
            nc.scalar.activation(tau[:], t_cur[:], ACT.Ln)
            if k < K:
                nc.vector.tensor_add(s_acc[:], s_acc[:], tau[:])
                nc.scalar.mul(s_acc[:], s_acc[:], 2.0)
            inv = sm_pool.tile([P, 1], F32, tag="inv")
            nc.vector.reciprocal(inv[:], t_cur[:])
            inv2 = sm_pool.tile([P, 1], F32, tag="inv2")
            nc.vector.tensor_tensor(out=inv2[:], in0=inv[:], in1=inv[:],
                                    op=ALU.mult)
            # t_next = F_tot / t^2
            nc.vector.tensor_tensor(out=t_cur[:], in0=totals[:, 0:1],
                                    in1=inv2[:], op=ALU.mult)
            if k == K:
                tau2 = sm_pool.tile([P, 1], F32, tag="tau2")
                nc.scalar.activation(tau2[:], t_cur[:], ACT.Ln)
                res = sm_pool.tile([P, 1], F32, tag="chainres")
                nc.vector.tensor_add(res[:], s_acc[:], tau[:])
                nc.vector.tensor_add(res[:], res[:], tau2[:])
                nc.scalar.mul(res[:], res[:], 1.0 / (2 ** K))
                return res
            xnew = pan_pool.tile([P, cw], mm_dt, tag="pan")

            def evict_scaled(m, psum_ap, dst=xnew, sc=inv2):
                sl = slice(m * pw, (m + 1) * pw)
                nc.scalar.activation(dst[:, sl], psum_ap, ACT.Copy,
                                     scale=sc[:])
            mm_pass(ag_out[:], xpan, evict_scaled)
            xpan = xnew
            f_rep = fnorm_partial(xpan)
        raise AssertionError("unreachable")

    # ---------- chain 1: lambda_max of M ----------
    trM_loc = diag_partial(mpan)

    def init_t_chain1(totals):
        nc.vector.tensor_copy(t_cur[:], totals[:, 1:2])
        nc.vector.tensor_copy(trMg[:], totals[:, 1:2])

    if debug_stage == "T0":
        f_rep = fnorm_partial(mpan)
        tail = make_tail(f_rep, trM_loc)
        ag_out, totals = do_allgather(mpan, tail)
        init_t_chain1(totals)
        tau = sm_pool.tile([P, 1], F32, tag="tau")
        nc.scalar.activation(tau[:], t_cur[:], ACT.Ln)
        inv = sm_pool.tile([P, 1], F32, tag="inv")
        nc.vector.reciprocal(inv[:], t_cur[:])
        dbg = sm_pool.tile([1, 8], F32, tag="dbgv")
        nc.vector.memset(dbg[:], 0.0)
        nc.vector.tensor_copy(dbg[:, 0:1], t_cur[0:1, :])
        nc.vector.tensor_copy(dbg[:, 1:2], totals[0:1, 0:1])
        nc.vector.tensor_copy(dbg[:, 2:3], tau[0:1, :])
        nc.vector.tensor_copy(dbg[:, 3:4], inv[0:1, :])
        nc.vector.tensor_copy(dbg[:, 4:5], f_rep[0:1, :])
        nc.vector.tensor_copy(dbg[:, 5:6], trM_loc[0:1, :])
        nc.sync.dma_start(dbg_out[:], dbg[:])
        z = sm_pool.tile([1, 2], F32, tag="tail")
        nc.vector.memset(z[:], 0.0)
        nc.sync.dma_start(loss_out[:], z[0:1, 0:1])
        return

    res1 = chain(mpan, k1, init_t_chain1, aux0=trM_loc)
    nc.vector.tensor_copy(ln_lam1[:], res1[:])

    if debug_stage == "C1":
        dbg = sm_pool.tile([1, 8], F32, tag="dbgv")
        nc.vector.memset(dbg[:], 0.0)
        nc.vector.tensor_copy(dbg[:, 0:1], ln_lam1[0:1, :])
        nc.vector.tensor_copy(dbg[:, 1:2], trMg[0:1, :])
        nc.sync.dma_start(dbg_out[:], dbg[:])
        z = sm_pool.tile([1, 2], F32, tag="tail")
        nc.vector.memset(z[:], 0.0)
        nc.sync.dma_start(loss_out[:], z[0:1, 0:1])
        return

    # ---------- chain 2: mu - lambda_min via B = mu I - M ----------
    nc.scalar.activation(mu[:], ln_lam1[:], ACT.Exp)
    nc.scalar.mul(mu[:], mu[:], MU_FACTOR)

    mpan2 = pan_pool.tile([P, cw], mm_dt, tag="pan")
    nc.sync.dma_start(mpan2[:], m_store[:])
    bpan = pan_pool.tile([P, cw], mm_dt, tag="pan")
    nc.vector.tensor_scalar_mul(bpan[:], ei[:], mu[:])
    nc.vector.tensor_tensor(out=bpan[:], in0=bpan[:], in1=mpan2[:],
                            op=ALU.subtract)

    def init_t_chain2(totals):
        # t0 = n*mu - tr(M)
        nc.scalar.mul(t_cur[:], mu[:], float(n))
        nc.vector.tensor_tensor(out=t_cur[:], in0=t_cur[:],
                                in1=trMg[:], op=ALU.subtract)

    res2 = chain(bpan, k2, init_t_chain2)

    # ---------- final scalar math ----------
    bmax = sm_pool.tile([P, 1], F32, tag="bmax")
    nc.scalar.activation(bmax[:], res2[:], ACT.Exp)
    lam_min = sm_pool.tile([P, 1], F32, tag="lammin")
    nc.vector.tensor_tensor(out=lam_min[:], in0=mu[:], in1=bmax[:],
                            op=ALU.subtract)
    ln_min = sm_pool.tile([P, 1], F32, tag="lnmin")
    nc.scalar.activation(ln_min[:], lam_min[:], ACT.Ln)
    loss = sm_pool.tile([P, 1], F32, tag="lossv")
    nc.vector.tensor_tensor(out=loss[:], in0=ln_lam1[:], in1=ln_min[:],
                            op=ALU.subtract)
    nc.sync.dma_start(loss_out[:], loss[0:1, :])

    dbg = sm_pool.tile([1, 8], F32, tag="dbgv")
    nc.vector.tensor_copy(dbg[:, 0:1], ln_lam1[0:1, :])
    nc.vector.tensor_copy(dbg[:, 1:2], mu[0:1, :])
    nc.vector.tensor_copy(dbg[:, 2:3], bmax[0:1, :])
    nc.vector.tensor_copy(dbg[:, 3:4], lam_min[0:1, :])
    nc.vector.tensor_copy(dbg[:, 4:5], trMg[0:1, :])
    nc.vector.tensor_copy(dbg[:, 5:6], loss[0:1, :])
    nc.sync.dma_start(dbg_out[:], dbg[:])


_NC_CACHE = {}


def _get_nc(n=2048, k1=K1, k2=K2, mm_dt=F32):
    key = (n, k1, k2, str(mm_dt))
    if key not in _NC_CACHE:
        _NC_CACHE[key] = _build_nc(n, k1, k2, mm_dt)
    return _NC_CACHE[key]


def _panelize(mat, i, n):
    """[128, (n//128)*(n//8)] panel of mat[:, i*pw:(i+1)*pw] in SBUF chunk
    layout pan[p, c*pw+j] = mat[c*128+p, i*pw+j]."""
    pw = n // N_CORES
    ch = n // P
    x = mat[:, i * pw:(i + 1) * pw].reshape(ch, P, pw)
    return np.ascontiguousarray(x.transpose(1, 0, 2).reshape(P, ch * pw))


def _pan_full(mat, n):
    """Rank-stacked panelized layout [8*(128+1), cw] matching AG output."""
    pw = n // N_CORES
    ch = n // P
    agr = P + 1
    out = np.zeros((N_CORES * agr, ch * pw), dtype=np.float32)
    for i in range(N_CORES):
        out[i * agr:i * agr + P, :] = _panelize(mat, i, n)
    return out


def _prep_inputs(pred_values, active_scales, A_factor, factor_rows,
                 factor_cols, n):
    G = np.asarray(A_factor, dtype=np.float32)
    GT = np.ascontiguousarray(G.T)
    vals = (np.asarray(pred_values, dtype=np.float32)
            * np.asarray(active_scales, dtype=np.float32))
    LT = np.eye(n, dtype=np.float32)
    np.add.at(LT, (np.asarray(factor_cols), np.asarray(factor_rows)), vals)
    eye = np.eye(n, dtype=np.float32)
    gt_pan_full = _pan_full(GT, n)
    lt_pan_full = _pan_full(LT, n)
    in_maps = []
    for i in range(N_CORES):
        in_maps.append({
            "gt_pan_full": gt_pan_full,
            "lt_pan_full": lt_pan_full,
            "git_pan": _panelize(GT, i, n),
            "lti_pan": _panelize(LT, i, n),
            "ei_pan": _panelize(eye, i, n),
        })
    return in_maps


def kernel(pred_values, active_scales, A_factor, factor_rows, factor_cols):
    n = A_factor.shape[0]
    nc = _get_nc(n=n)
    in_maps = _prep_inputs(pred_values, active_scales, A_factor,
                           factor_rows, factor_cols, n)
    res = run_bass_kernel_spmd(nc, in_maps, core_ids=list(range(N_CORES)))
    out = res.results[0]["loss"]
    return np.float32(out[0, 0])


if __name__ == "__main__":
    import reference, jax
    cpu = jax.devices("cpu")[0]
    with jax.default_device(cpu):
        inputs = {k: np.asarray(v) for k, v in reference.setup_inputs().items()}
    got = kernel(**inputs)
    print("kernel loss:", got)


# revision 19
# speedup vs baseline: 5.3274x; 5.3274x over previous
"""Trainium2 Bass kernel for nn_CachedConditionNumberLoss.

Computes loss = log(lambda_max) - log(lambda_min) of M = L A L^T where
A = G G^T/n + I  (G = A_factor, n = 2048) and L = I + scatter(pred*scale).

Strategy (8-core SPMD, column-panel sharded):
  - core i owns the column panel X[:, i*PW:(i+1)*PW] of every 2048x2048
    matrix involved; all cross-core exchange is AllGather of panels.
  - device computes A = G G^T/n + I, W = A L^T, M = L W (three panel
    matmul passes), then extremal eigenvalues of M via two repeated-
    squaring chains with trace-ratio estimators:
      chain 1 on M          -> lambda_max
      chain 2 on mu*I - M   -> mu - lambda_min   (mu = 1.001*lambda_max)
    Each squaring step: AllGather panels of X_k (with the Frobenius-norm
    partial embedded in a tail row), then each core computes
    X_{k+1}[:, panel] = (X_k^T X_k)[:, panel] / t_k^2 with PE matmuls
    (lhsT tiles are read straight from the gathered copy; symmetry of X_k
    makes transposes unnecessary).  Trace bookkeeping on device:
      t_{k+1} = ||X_k||_F^2 / t_k^2,  tau_k = ln t_k,
      s_{k+1} = 2 (s_k + tau_k),
      ln(lam_hat) = (s_K + tau_K + tau_{K+1}) / 2^K.
  - host only transposes/slices inputs (incl. assembling L^T from the
    scatter triplets) and reads back the scalar.
"""

import numpy as np

import concourse.tile as tile
from concourse import bacc, mybir
from concourse.bass_utils import run_bass_kernel_spmd

F32 = mybir.dt.float32
ACT = mybir.ActivationFunctionType
ALU = mybir.AluOpType
P = 128
N_CORES = 8

# squaring-chain lengths (validated in fp32 simulation: loss relerr ~2e-7)
K1 = 11
K2 = 14
MU_FACTOR = 1.001


def _build_nc(n=2048, k1=K1, k2=K2, mm_dt=F32, debug_stage=None):
    ch = n // P           # 128-row chunks per matrix (16)
    pw = n // N_CORES     # panel width per core (256)
    cw = ch * pw          # panel free size in SBUF layout (4096)
    agr = P + 1           # rows per rank in AG buffers (tail row at P)
    cpp = pw // P         # column chunks per panel (2)

    nc = bacc.Bacc(None, target_bir_lowering=False)

    git_pan = nc.dram_tensor("git_pan", [P, cw], F32, kind="ExternalInput")
    lti_pan = nc.dram_tensor("lti_pan", [P, cw], F32, kind="ExternalInput")
    ei_pan = nc.dram_tensor("ei_pan", [P, cw], F32, kind="ExternalInput")

    loss_out = nc.dram_tensor("loss", [1, 1], F32, kind="ExternalOutput")
    dbg_out = nc.dram_tensor("dbg", [1, 8], F32, kind="ExternalOutput")

    m_store = nc.dram_tensor("m_store", [P, cw], F32, kind="Internal")
    pan_out = (nc.dram_tensor("pan_out", [P, cw], F32, kind="ExternalOutput")
               if debug_stage in ("A", "W", "M") else None)

    with tile.TileContext(nc) as tc:
        with (
            tc.tile_pool(name="xf", bufs=7) as xf_pool,
            tc.tile_pool(name="pan", bufs=3) as pan_pool,
            tc.tile_pool(name="eip", bufs=1) as ei_pool,
            tc.tile_pool(name="small", bufs=6) as sm_pool,
            tc.tile_pool(name="state", bufs=1) as st_pool,
            tc.tile_pool(name="psum", bufs=6, space="PSUM") as ps_pool,
            tc.tile_pool(name="psr", bufs=2, space="PSUM") as psr_pool,
            tc.tile_pool(name="dram", bufs=2, space="DRAM") as dram_pool,
        ):
            _trace_program(
                nc, n, k1, k2, mm_dt, debug_stage,
                ch, pw, cw, agr, cpp,
                git_pan, lti_pan, ei_pan,
                loss_out, dbg_out, m_store, pan_out,
                xf_pool, pan_pool, ei_pool, sm_pool, st_pool,
                ps_pool, psr_pool, dram_pool,
            )

    nc.compile()
    return nc


def _trace_program(nc, n, k1, k2, mm_dt, debug_stage,
                   ch, pw, cw, agr, cpp,
                   git_pan, lti_pan, ei_pan,
                   loss_out, dbg_out, m_store, pan_out,
                   xf_pool, pan_pool, ei_pool, sm_pool, st_pool,
                   ps_pool, psr_pool, dram_pool):
    ones = st_pool.tile([P, P], F32)
    nc.vector.memset(ones[:], 1.0)

    ei = ei_pool.tile([P, cw], F32, tag="ei")
    nc.sync.dma_start(ei[:], ei_pan[:])

    # ---------- helpers ----------
    def part_reduce(vec_ap, width=1):
        """[p, width] -> [P, width] replicated column sums."""
        red = psr_pool.tile([P, 2], F32, space="PSUM", tag="red")
        p_sz = vec_ap.shape[0]
        nc.tensor.matmul(red[:, 0:width], lhsT=ones[:p_sz, :],
                         rhs=vec_ap, start=True, stop=True)
        out = sm_pool.tile([P, width], F32, tag="pred")
        nc.vector.tensor_copy(out[:], red[:, 0:width])
        return out

    def fnorm_partial(pan_tile):
        """sum of squares of a [P, cw] panel -> [P,1] replicated."""
        acc = sm_pool.tile([P, ch], F32, tag="facc")
        for c in range(ch):
            tmp = sm_pool.tile([P, pw], F32, tag="sqtmp")
            nc.scalar.activation(tmp[:], pan_tile[:, c * pw:(c + 1) * pw],
                                 ACT.Square, accum_out=acc[:, c:c + 1])
        accs = sm_pool.tile([P, 1], F32, tag="faccs")
        nc.vector.reduce_sum(accs[:], acc[:], axis=mybir.AxisListType.X)
        return part_reduce(accs[:])

    def diag_partial(pan_tile):
        """sum of (panel .* ei) -> [P,1] replicated local trace partial."""
        acc = sm_pool.tile([P, ch], F32, tag="facc")
        for c in range(ch):
            sl = slice(c * pw, (c + 1) * pw)
            tmp = sm_pool.tile([P, pw], F32, tag="sqtmp")
            nc.vector.tensor_tensor(out=tmp[:], in0=pan_tile[:, sl],
                                    in1=ei[:, sl], op=ALU.mult)
            nc.vector.reduce_sum(acc[:, c:c + 1], tmp[:],
                                 axis=mybir.AxisListType.X)
        accs = sm_pool.tile([P, 1], F32, tag="faccs")
        nc.vector.reduce_sum(accs[:], acc[:], axis=mybir.AxisListType.X)
        return part_reduce(accs[:])

    def mm_pass(src_dram, rhs_tile, evict_fn):
        """out[:, panel] = X^T @ rhs_panel, X stored panelized in src_dram."""
        tiles = []
        for r in range(N_CORES):
            t = xf_pool.tile([P, cw], mm_dt, tag="xf")
            nc.sync.dma_start(t[:], src_dram[r * agr:r * agr + P, :])
            tiles.append(t)
        for m in range(ch):
            acc = ps_pool.tile([P, pw], F32, space="PSUM", tag="mm")
            t = tiles[m // cpp]
            base = (m % cpp) * P
            for k in range(ch):
                nc.tensor.matmul(
                    acc[:],
                    lhsT=t[:, k * pw + base:k * pw + base + P],
                    rhs=rhs_tile[:, k * pw:(k + 1) * pw],
                    start=(k == 0), stop=(k == ch - 1),
                )
            evict_fn(m, acc[:])

    def do_allgather(pan_tile, tail_tile):
        """AG panels + 2-value tail; returns (ag_out, totals[P,2])."""
        ag_in = dram_pool.tile([agr, cw], F32, tag="agin")
        ag_out = dram_pool.tile([N_CORES * agr, cw], F32, tag="agout",
                                addr_space="Shared")
        nc.sync.dma_start(ag_in[0:P, :], pan_tile[:])
        nc.sync.dma_start(ag_in[P:P + 1, 0:2], tail_tile[0:1, 0:2])
        nc.gpsimd.collective_compute(
            "AllGather", ALU.bypass,
            ins=[ag_in[:]], outs=[ag_out[:]],
            replica_groups=[list(range(N_CORES))],
        )
        tails8 = sm_pool.tile([N_CORES, 2], F32, tag="tails8")
        nc.sync.dma_start(
            tails8[:],
            ag_out.rearrange("(r p) c -> r p c", p=agr)[:, P:P + 1, 0:2])
        totals = part_reduce(tails8[:], width=2)
        return ag_out, totals

    def tiny_allgather(tail_tile):
        agt_in = dram_pool.tile([1, 16], F32, tag="agtin")
        agt_out = dram_pool.tile([N_CORES, 16], F32, tag="agtout",
                                 addr_space="Shared")
        pad = sm_pool.tile([1, 16], F32, tag="tailpad")
        nc.vector.memset(pad[:], 0.0)
        nc.vector.tensor_copy(pad[:, 0:2], tail_tile[0:1, 0:2])
        nc.sync.dma_start(agt_in[:], pad[:])
        nc.gpsimd.collective_compute(
            "AllGather", ALU.bypass,
            ins=[agt_in[:]], outs=[agt_out[:]],
            replica_groups=[list(range(N_CORES))],
        )
        t8 = sm_pool.tile([N_CORES, 2], F32, tag="tails8")
        nc.sync.dma_start(t8[:], agt_out[:, 0:2])
        return part_reduce(t8[:], width=2)

    def make_tail(f_rep, aux_rep=None):
        t = sm_pool.tile([1, 2], F32, tag="tail")
        nc.vector.tensor_copy(t[:, 0:1], f_rep[0:1, :])
        if aux_rep is not None:
            nc.vector.tensor_copy(t[:, 1:2], aux_rep[0:1, :])
        else:
            nc.vector.memset(t[:, 1:2], 0.0)
        return t

    def _dbg_finish(tile_):
        nc.sync.dma_start(pan_out[:], tile_[:])
        z = sm_pool.tile([1, 2], F32, tag="tail")
        nc.vector.memset(z[:], 0.0)
        nc.sync.dma_start(loss_out[:], z[0:1, 0:1])
        d = sm_pool.tile([1, 8], F32, tag="dbgv")
        nc.vector.memset(d[:], 0.0)
        nc.sync.dma_start(dbg_out[:], d[:])

    # ---------- formation: A = G G^T / n + I ----------
    gpan = pan_pool.tile([P, cw], mm_dt, tag="pan")
    nc.sync.dma_start(gpan[:], git_pan[:])

    zt0 = sm_pool.tile([1, 2], F32, tag="tail")
    nc.vector.memset(zt0[:], 0.0)
    agG, _ = do_allgather(gpan, zt0)         # full G^T, panelized

    # L^T panels: AG early too (full L^T needed for the M pass)
    ltpan = pan_pool.tile([P, cw], mm_dt, tag="pan")
    nc.sync.dma_start(ltpan[:], lti_pan[:])
    zt1 = sm_pool.tile([1, 2], F32, tag="tail")
    nc.vector.memset(zt1[:], 0.0)
    agLT, _ = do_allgather(ltpan, zt1)       # full L^T, panelized

    apan = pan_pool.tile([P, cw], mm_dt, tag="pan")

    def evict_a(m, psum_ap):
        sl = slice(m * pw, (m + 1) * pw)
        nc.scalar.activation(apan[:, sl], psum_ap, ACT.Copy, scale=1.0 / n)
        nc.vector.tensor_add(apan[:, sl], apan[:, sl], ei[:, sl])

    mm_pass(agG[:], gpan, evict_a)

    if debug_stage == "A":
        _dbg_finish(apan)
        return

    # ---------- AG(A); W = A L^T ----------
    zt = sm_pool.tile([1, 2], F32, tag="tail")
    nc.vector.memset(zt[:], 0.0)
    agA, _ = do_allgather(apan, zt)

    wpan = pan_pool.tile([P, cw], mm_dt, tag="pan")

    def evict_plain(dst):
        def fn(m, psum_ap):
            sl = slice(m * pw, (m + 1) * pw)
            nc.scalar.activation(dst[:, sl], psum_ap, ACT.Copy)
        return fn

    mm_pass(agA[:], ltpan, evict_plain(wpan))

    if debug_stage == "W":
        _dbg_finish(wpan)
        return

    # ---------- M = L W ----------
    mpan = pan_pool.tile([P, cw], mm_dt, tag="pan")
    mm_pass(agLT[:], wpan, evict_plain(mpan))
    nc.sync.dma_start(m_store[:], mpan[:])

    if debug_stage == "M":
        _dbg_finish(mpan)
        return

    # persistent chain state
    t_cur = st_pool.tile([P, 1], F32)
    s_acc = st_pool.tile([P, 1], F32)
    ln_lam1 = st_pool.tile([P, 1], F32)
    mu = st_pool.tile([P, 1], F32)
    trMg = st_pool.tile([P, 1], F32)   # global trace of M

    def chain(x0_tile, K, init_t_fn, aux0=None):
        """Squaring chain; returns ln(lam_hat) as a [P,1] tile."""
        nc.vector.memset(s_acc[:], 0.0)
        xpan = x0_tile
        f_rep = fnorm_partial(xpan)
        for k in range(K + 1):
            tail = make_tail(f_rep, aux0 if k == 0 else None)
            if k < K:
                ag_out, totals = do_allgather(xpan, tail)
            else:
                totals = tiny_allgather(tail)
            if k == 0:
                init_t_fn(totals)
            tau = sm_pool.tile([P, 1], F32, tag="tau")
            nc.scalar.activation(tau[:], t_cur[:], ACT.Ln)
            if k < K:
                nc.vector.tensor_add(s_acc[:], s_acc[:], tau[:])
                nc.scalar.mul(s_acc[:], s_acc[:], 2.0)
            inv = sm_pool.tile([P, 1], F32, tag="inv")
            nc.vector.reciprocal(inv[:], t_cur[:])
            inv2 = sm_pool.tile([P, 1], F32, tag="inv2")
            nc.vector.tensor_tensor(out=inv2[:], in0=inv[:], in1=inv[:],
                                    op=ALU.mult)
            # t_next = F_tot / t^2
            nc.vector.tensor_tensor(out=t_cur[:], in0=totals[:, 0:1],
                                    in1=inv2[:], op=ALU.mult)
            if k == K:
                tau2 = sm_pool.tile([P, 1], F32, tag="tau2")
                nc.scalar.activation(tau2[:], t_cur[:], ACT.Ln)
                res = sm_pool.tile([P, 1], F32, tag="chainres")
                nc.vector.tensor_add(res[:], s_acc[:], tau[:])
                nc.vector.tensor_add(res[:], res[:], tau2[:])
                nc.scalar.mul(res[:], res[:], 1.0 / (2 ** K))
                return res
            xnew = pan_pool.tile([P, cw], mm_dt, tag="pan")

            def evict_scaled(m, psum_ap, dst=xnew, sc=inv2):
                sl = slice(m * pw, (m + 1) * pw)
                nc.scalar.activation(dst[:, sl], psum_ap, ACT.Copy,
                                     scale=sc[:])
            mm_pass(ag_out[:], xpan, evict_scaled)
            xpan = xnew
            f_rep = fnorm_partial(xpan)
        raise AssertionError("unreachable")

    # ---------- chain 1: lambda_max of M ----------
    trM_loc = diag_partial(mpan)

    def init_t_chain1(totals):
        nc.vector.tensor_copy(t_cur[:], totals[:, 1:2])
        nc.vector.tensor_copy(trMg[:], totals[:, 1:2])

    if debug_stage == "T0":
        f_rep = fnorm_partial(mpan)
        tail = make_tail(f_rep, trM_loc)
        ag_out, totals = do_allgather(mpan, tail)
        init_t_chain1(totals)
        tau = sm_pool.tile([P, 1], F32, tag="tau")
        nc.scalar.activation(tau[:], t_cur[:], ACT.Ln)
        inv = sm_pool.tile([P, 1], F32, tag="inv")
        nc.vector.reciprocal(inv[:], t_cur[:])
        dbg = sm_pool.tile([1, 8], F32, tag="dbgv")
        nc.vector.memset(dbg[:], 0.0)
        nc.vector.tensor_copy(dbg[:, 0:1], t_cur[0:1, :])
        nc.vector.tensor_copy(dbg[:, 1:2], totals[0:1, 0:1])
        nc.vector.tensor_copy(dbg[:, 2:3], tau[0:1, :])
        nc.vector.tensor_copy(dbg[:, 3:4], inv[0:1, :])
        nc.vector.tensor_copy(dbg[:, 4:5], f_rep[0:1, :])
        nc.vector.tensor_copy(dbg[:, 5:6], trM_loc[0:1, :])
        nc.sync.dma_start(dbg_out[:], dbg[:])
        z = sm_pool.tile([1, 2], F32, tag="tail")
        nc.vector.memset(z[:], 0.0)
        nc.sync.dma_start(loss_out[:], z[0:1, 0:1])
        return

    res1 = chain(mpan, k1, init_t_chain1, aux0=trM_loc)
    nc.vector.tensor_copy(ln_lam1[:], res1[:])

    if debug_stage == "C1":
        dbg = sm_pool.tile([1, 8], F32, tag="dbgv")
        nc.vector.memset(dbg[:], 0.0)
        nc.vector.tensor_copy(dbg[:, 0:1], ln_lam1[0:1, :])
        nc.vector.tensor_copy(dbg[:, 1:2], trMg[0:1, :])
        nc.sync.dma_start(dbg_out[:], dbg[:])
        z = sm_pool.tile([1, 2], F32, tag="tail")
        nc.vector.memset(z[:], 0.0)
        nc.sync.dma_start(loss_out[:], z[0:1, 0:1])
        return

    # ---------- chain 2: mu - lambda_min via B = mu I - M ----------
    nc.scalar.activation(mu[:], ln_lam1[:], ACT.Exp)
    nc.scalar.mul(mu[:], mu[:], MU_FACTOR)

    mpan2 = pan_pool.tile([P, cw], mm_dt, tag="pan")
    nc.sync.dma_start(mpan2[:], m_store[:])
    bpan = pan_pool.tile([P, cw], mm_dt, tag="pan")
    nc.vector.tensor_scalar_mul(bpan[:], ei[:], mu[:])
    nc.vector.tensor_tensor(out=bpan[:], in0=bpan[:], in1=mpan2[:],
                            op=ALU.subtract)

    def init_t_chain2(totals):
        # t0 = n*mu - tr(M)
        nc.scalar.mul(t_cur[:], mu[:], float(n))
        nc.vector.tensor_tensor(out=t_cur[:], in0=t_cur[:],
                                in1=trMg[:], op=ALU.subtract)

    res2 = chain(bpan, k2, init_t_chain2)

    # ---------- final scalar math ----------
    bmax = sm_pool.tile([P, 1], F32, tag="bmax")
    nc.scalar.activation(bmax[:], res2[:], ACT.Exp)
    lam_min = sm_pool.tile([P, 1], F32, tag="lammin")
    nc.vector.tensor_tensor(out=lam_min[:], in0=mu[:], in1=bmax[:],
                            op=ALU.subtract)
    ln_min = sm_pool.tile([P, 1], F32, tag="lnmin")
    nc.scalar.activation(ln_min[:], lam_min[:], ACT.Ln)
    loss = sm_pool.tile([P, 1], F32, tag="lossv")
    nc.vector.tensor_tensor(out=loss[:], in0=ln_lam1[:], in1=ln_min[:],
                            op=ALU.subtract)
    nc.sync.dma_start(loss_out[:], loss[0:1, :])

    dbg = sm_pool.tile([1, 8], F32, tag="dbgv")
    nc.vector.tensor_copy(dbg[:, 0:1], ln_lam1[0:1, :])
    nc.vector.tensor_copy(dbg[:, 1:2], mu[0:1, :])
    nc.vector.tensor_copy(dbg[:, 2:3], bmax[0:1, :])
    nc.vector.tensor_copy(dbg[:, 3:4], lam_min[0:1, :])
    nc.vector.tensor_copy(dbg[:, 4:5], trMg[0:1, :])
    nc.vector.tensor_copy(dbg[:, 5:6], loss[0:1, :])
    nc.sync.dma_start(dbg_out[:], dbg[:])


_NC_CACHE = {}


def _get_nc(n=2048, k1=K1, k2=K2, mm_dt=F32):
    key = (n, k1, k2, str(mm_dt))
    if key not in _NC_CACHE:
        _NC_CACHE[key] = _build_nc(n, k1, k2, mm_dt)
    return _NC_CACHE[key]


def _panelize(mat, i, n):
    """[128, (n//128)*(n//8)] panel of mat[:, i*pw:(i+1)*pw] in SBUF chunk
    layout pan[p, c*pw+j] = mat[c*128+p, i*pw+j]."""
    pw = n // N_CORES
    ch = n // P
    x = mat[:, i * pw:(i + 1) * pw].reshape(ch, P, pw)
    return np.ascontiguousarray(x.transpose(1, 0, 2).reshape(P, ch * pw))


def _pan_full(mat, n):
    """Rank-stacked panelized layout [8*(128+1), cw] matching AG output."""
    pw = n // N_CORES
    ch = n // P
    agr = P + 1
    out = np.zeros((N_CORES * agr, ch * pw), dtype=np.float32)
    for i in range(N_CORES):
        out[i * agr:i * agr + P, :] = _panelize(mat, i, n)
    return out


def _prep_inputs(pred_values, active_scales, A_factor, factor_rows,
                 factor_cols, n):
    G = np.asarray(A_factor, dtype=np.float32)
    GT = np.ascontiguousarray(G.T)
    vals = (np.asarray(pred_values, dtype=np.float32)
            * np.asarray(active_scales, dtype=np.float32))
    LT = np.eye(n, dtype=np.float32)
    np.add.at(LT, (np.asarray(factor_cols), np.asarray(factor_rows)), vals)
    eye = np.eye(n, dtype=np.float32)
    in_maps = []
    for i in range(N_CORES):
        in_maps.append({
            "git_pan": _panelize(GT, i, n),
            "lti_pan": _panelize(LT, i, n),
            "ei_pan": _panelize(eye, i, n),
        })
    return in_maps


_RUNNER_CACHE = {}


def _make_pjrt_runner(nc):
    """Cached jit(shard_map) runner for the axon/PJRT path: avoids the
    per-call retrace that run_bass_via_pjrt pays, so repeat kernel() calls
    cost transfer + execute only."""
    import jax
    from jax.sharding import Mesh, PartitionSpec
    try:
        from jax.experimental.shard_map import shard_map
    except Exception:
        from jax.shard_map import shard_map  # newer jax
    from concourse import bass2jax
    from concourse import mybir as _mybir

    bass2jax.install_neuronx_cc_hook()
    partition_name = (nc.partition_id_tensor.name
                      if nc.partition_id_tensor else None)
    in_names, out_names, out_avals, zero_shapes = [], [], [], []
    for alloc in nc.m.functions[0].allocations:
        if not isinstance(alloc, _mybir.MemoryLocationSet):
            continue
        name = alloc.memorylocations[0].name
        if alloc.kind == "ExternalInput":
            if name != partition_name:
                in_names.append(name)
        elif alloc.kind == "ExternalOutput":
            out_names.append(name)
            shape = tuple(alloc.tensor_shape)
            dtype = _mybir.dt.np(alloc.dtype)
            out_avals.append(jax.core.ShapedArray(shape, dtype))
            zero_shapes.append((shape, dtype))
    n_params = len(in_names)
    all_in_names = list(in_names) + list(out_names)
    if partition_name is not None:
        all_in_names.append(partition_name)
    donate = tuple(range(n_params, n_params + len(out_names)))

    def _body(*args):
        operands = list(args)
        if partition_name is not None:
            operands.append(bass2jax.partition_id_tensor())
        outs = bass2jax._bass_exec_p.bind(
            *operands,
            out_avals=tuple(out_avals),
            in_names=tuple(all_in_names),
            out_names=tuple(out_names),
            lowering_input_output_aliases=(),
            sim_require_finite=True,
            sim_require_nnan=True,
            nc=nc,
        )
        return tuple(outs)

    devices = jax.devices()[:N_CORES]
    mesh = Mesh(np.asarray(devices), ("core",))
    n_args = n_params + len(out_names)
    sharded = jax.jit(
        shard_map(_body, mesh=mesh,
                  in_specs=(PartitionSpec("core"),) * n_args,
                  out_specs=(PartitionSpec("core"),) * len(out_names),
                  check_rep=False),
        donate_argnums=donate, keep_unused=True)

    def run(in_maps):
        concat_in = [
            np.concatenate([np.asarray(in_maps[c][nm]) for c in range(N_CORES)],
                           axis=0)
            for nm in in_names
        ]
        concat_zeros = [
            np.zeros((N_CORES * s[0],) + tuple(s[1:]), dt)
            for (s, dt) in zero_shapes
        ]
        out_arrs = sharded(*concat_in, *concat_zeros)
        res = []
        for c in range(N_CORES):
            res.append({
                nm: np.asarray(out_arrs[i]).reshape(
                    N_CORES, *out_avals[i].shape)[c]
                for i, nm in enumerate(out_names)
            })
        return res

    return run


def _run(nc, in_maps):
    from concourse._compat import axon_active
    if axon_active():
        key = id(nc)
        if key not in _RUNNER_CACHE:
            _RUNNER_CACHE[key] = _make_pjrt_runner(nc)
        return _RUNNER_CACHE[key](in_maps)
    return run_bass_kernel_spmd(
        nc, in_maps, core_ids=list(range(N_CORES))).results


def kernel(pred_values, active_scales, A_factor, factor_rows, factor_cols):
    n = A_factor.shape[0]
    nc = _get_nc(n=n)
    in_maps = _prep_inputs(pred_values, active_scales, A_factor,
                           factor_rows, factor_cols, n)
    results = _run(nc, in_maps)
    out = results[0]["loss"]
    return np.float32(out[0, 0])


if __name__ == "__main__":
    import reference, jax
    cpu = jax.devices("cpu")[0]
    with jax.default_device(cpu):
        inputs = {k: np.asarray(v) for k, v in reference.setup_inputs().items()}
    got = kernel(**inputs)
    print("kernel loss:", got)


# revision 21
# speedup vs baseline: 7.0566x; 1.3246x over previous
"""Trainium2 Bass kernel for nn_CachedConditionNumberLoss.

Computes loss = log(lambda_max) - log(lambda_min) of M = L A L^T where
A = G G^T/n + I  (G = A_factor, n = 2048) and L = I + scatter(pred*scale).

Strategy (8-core SPMD, column-panel sharded):
  - core i owns the column panel X[:, i*PW:(i+1)*PW] of every 2048x2048
    matrix involved; all cross-core exchange is AllGather of panels.
  - device computes A = G G^T/n + I, W = A L^T, M = L W (three panel
    matmul passes), then extremal eigenvalues of M via two repeated-
    squaring chains with trace-ratio estimators:
      chain 1 on M          -> lambda_max
      chain 2 on mu*I - M   -> mu - lambda_min   (mu = 1.001*lambda_max)
    Each squaring step: AllGather panels of X_k (with the Frobenius-norm
    partial embedded in a tail row), then each core computes
    X_{k+1}[:, panel] = (X_k^T X_k)[:, panel] / t_k^2 with PE matmuls
    (lhsT tiles are read straight from the gathered copy; symmetry of X_k
    makes transposes unnecessary).  Trace bookkeeping on device:
      t_{k+1} = ||X_k||_F^2 / t_k^2,  tau_k = ln t_k,
      s_{k+1} = 2 (s_k + tau_k),
      ln(lam_hat) = (s_K + tau_K + tau_{K+1}) / 2^K.
  - host only transposes/slices inputs (incl. assembling L^T from the
    scatter triplets) and reads back the scalar.
"""

import numpy as np

import concourse.tile as tile
from concourse import bacc, mybir
from concourse.bass_utils import run_bass_kernel_spmd

F32 = mybir.dt.float32
ACT = mybir.ActivationFunctionType
ALU = mybir.AluOpType
P = 128
N_CORES = 8

# squaring-chain lengths (validated in fp32 simulation: loss relerr ~2e-7)
K1 = 11
K2 = 14
MU_FACTOR = 1.001


def _build_nc(n=2048, k1=K1, k2=K2, mm_dt=F32, debug_stage=None):
    ch = n // P           # 128-row chunks per matrix (16)
    pw = n // N_CORES     # panel width per core (256)
    cw = ch * pw          # panel free size in SBUF layout (4096)
    agr = P + 1           # rows per rank in AG buffers (tail row at P)
    cpp = pw // P         # column chunks per panel (2)

    nc = bacc.Bacc(None, target_bir_lowering=False)

    git_pan = nc.dram_tensor("git_pan", [P, cw], F32, kind="ExternalInput")
    lti_pan = nc.dram_tensor("lti_pan", [P, cw], F32, kind="ExternalInput")
    ei_pan = nc.dram_tensor("ei_pan", [P, cw], F32, kind="ExternalInput")

    loss_out = nc.dram_tensor("loss", [1, 1], F32, kind="ExternalOutput")
    dbg_out = nc.dram_tensor("dbg", [1, 8], F32, kind="ExternalOutput")

    m_store = nc.dram_tensor("m_store", [P, cw], F32, kind="Internal")
    pan_out = (nc.dram_tensor("pan_out", [P, cw], F32, kind="ExternalOutput")
               if debug_stage in ("A", "W", "M") else None)

    with tile.TileContext(nc) as tc:
        with (
            tc.tile_pool(name="xf", bufs=7) as xf_pool,
            tc.tile_pool(name="pan", bufs=3) as pan_pool,
            tc.tile_pool(name="eip", bufs=1) as ei_pool,
            tc.tile_pool(name="small", bufs=6) as sm_pool,
            tc.tile_pool(name="state", bufs=1) as st_pool,
            tc.tile_pool(name="psum", bufs=6, space="PSUM") as ps_pool,
            tc.tile_pool(name="psr", bufs=2, space="PSUM") as psr_pool,
            tc.tile_pool(name="dram", bufs=2, space="DRAM") as dram_pool,
        ):
            _trace_program(
                nc, n, k1, k2, mm_dt, debug_stage,
                ch, pw, cw, agr, cpp,
                git_pan, lti_pan, ei_pan,
                loss_out, dbg_out, m_store, pan_out,
                xf_pool, pan_pool, ei_pool, sm_pool, st_pool,
                ps_pool, psr_pool, dram_pool,
            )

    nc.compile()
    return nc


def _trace_program(nc, n, k1, k2, mm_dt, debug_stage,
                   ch, pw, cw, agr, cpp,
                   git_pan, lti_pan, ei_pan,
                   loss_out, dbg_out, m_store, pan_out,
                   xf_pool, pan_pool, ei_pool, sm_pool, st_pool,
                   ps_pool, psr_pool, dram_pool):
    ones = st_pool.tile([P, P], F32)
    nc.vector.memset(ones[:], 1.0)

    ei = ei_pool.tile([P, cw], F32, tag="ei")
    nc.sync.dma_start(ei[:], ei_pan[:])

    # ---------- helpers ----------
    def part_reduce(vec_ap, width=1):
        """[p, width] -> [P, width] replicated column sums."""
        red = psr_pool.tile([P, 2], F32, space="PSUM", tag="red")
        p_sz = vec_ap.shape[0]
        nc.tensor.matmul(red[:, 0:width], lhsT=ones[:p_sz, :],
                         rhs=vec_ap, start=True, stop=True)
        out = sm_pool.tile([P, width], F32, tag="pred")
        nc.vector.tensor_copy(out[:], red[:, 0:width])
        return out

    def fnorm_partial(pan_tile):
        """sum of squares of a [P, cw] panel -> [P,1] replicated."""
        acc = sm_pool.tile([P, ch], F32, tag="facc")
        for c in range(ch):
            tmp = sm_pool.tile([P, pw], F32, tag="sqtmp")
            nc.scalar.activation(tmp[:], pan_tile[:, c * pw:(c + 1) * pw],
                                 ACT.Square, accum_out=acc[:, c:c + 1])
        accs = sm_pool.tile([P, 1], F32, tag="faccs")
        nc.vector.reduce_sum(accs[:], acc[:], axis=mybir.AxisListType.X)
        return part_reduce(accs[:])

    def diag_partial(pan_tile):
        """sum of (panel .* ei) -> [P,1] replicated local trace partial."""
        acc = sm_pool.tile([P, ch], F32, tag="facc")
        for c in range(ch):
            sl = slice(c * pw, (c + 1) * pw)
            tmp = sm_pool.tile([P, pw], F32, tag="sqtmp")
            nc.vector.tensor_tensor(out=tmp[:], in0=pan_tile[:, sl],
                                    in1=ei[:, sl], op=ALU.mult)
            nc.vector.reduce_sum(acc[:, c:c + 1], tmp[:],
                                 axis=mybir.AxisListType.X)
        accs = sm_pool.tile([P, 1], F32, tag="faccs")
        nc.vector.reduce_sum(accs[:], acc[:], axis=mybir.AxisListType.X)
        return part_reduce(accs[:])

    f32r = (mm_dt == mybir.dt.float32r)

    def _mmcast(ap):
        return ap.bitcast(mybir.dt.float32r) if f32r else ap

    def mm_pass(src_dram, rhs_tile, evict_fn):
        """out[:, panel] = X^T @ rhs_panel, X stored panelized in src_dram."""
        tiles = []
        for r in range(N_CORES):
            t = xf_pool.tile([P, cw], F32, tag="xf")
            nc.sync.dma_start(t[:], src_dram[r * agr:r * agr + P, :])
            tiles.append(t)
        for m in range(ch):
            acc = ps_pool.tile([P, pw], F32, space="PSUM", tag="mm")
            t = tiles[m // cpp]
            base = (m % cpp) * P
            for k in range(ch):
                nc.tensor.matmul(
                    acc[:],
                    lhsT=_mmcast(t[:, k * pw + base:k * pw + base + P]),
                    rhs=_mmcast(rhs_tile[:, k * pw:(k + 1) * pw]),
                    start=(k == 0), stop=(k == ch - 1),
                )
            evict_fn(m, acc[:])

    def do_allgather(pan_tile, tail_tile):
        """AG panels + 2-value tail; returns (ag_out, totals[P,2])."""
        ag_in = dram_pool.tile([agr, cw], F32, tag="agin")
        ag_out = dram_pool.tile([N_CORES * agr, cw], F32, tag="agout",
                                addr_space="Shared")
        nc.sync.dma_start(ag_in[0:P, :], pan_tile[:])
        nc.sync.dma_start(ag_in[P:P + 1, 0:2], tail_tile[0:1, 0:2])
        nc.gpsimd.collective_compute(
            "AllGather", ALU.bypass,
            ins=[ag_in[:]], outs=[ag_out[:]],
            replica_groups=[list(range(N_CORES))],
        )
        tails8 = sm_pool.tile([N_CORES, 2], F32, tag="tails8")
        nc.sync.dma_start(
            tails8[:],
            ag_out.rearrange("(r p) c -> r p c", p=agr)[:, P:P + 1, 0:2])
        totals = part_reduce(tails8[:], width=2)
        return ag_out, totals

    def tiny_allgather(tail_tile):
        agt_in = dram_pool.tile([1, 16], F32, tag="agtin")
        agt_out = dram_pool.tile([N_CORES, 16], F32, tag="agtout",
                                 addr_space="Shared")
        pad = sm_pool.tile([1, 16], F32, tag="tailpad")
        nc.vector.memset(pad[:], 0.0)
        nc.vector.tensor_copy(pad[:, 0:2], tail_tile[0:1, 0:2])
        nc.sync.dma_start(agt_in[:], pad[:])
        nc.gpsimd.collective_compute(
            "AllGather", ALU.bypass,
            ins=[agt_in[:]], outs=[agt_out[:]],
            replica_groups=[list(range(N_CORES))],
        )
        t8 = sm_pool.tile([N_CORES, 2], F32, tag="tails8")
        nc.sync.dma_start(t8[:], agt_out[:, 0:2])
        return part_reduce(t8[:], width=2)

    def make_tail(f_rep, aux_rep=None):
        t = sm_pool.tile([1, 2], F32, tag="tail")
        nc.vector.tensor_copy(t[:, 0:1], f_rep[0:1, :])
        if aux_rep is not None:
            nc.vector.tensor_copy(t[:, 1:2], aux_rep[0:1, :])
        else:
            nc.vector.memset(t[:, 1:2], 0.0)
        return t

    def _dbg_finish(tile_):
        nc.sync.dma_start(pan_out[:], tile_[:])
        z = sm_pool.tile([1, 2], F32, tag="tail")
        nc.vector.memset(z[:], 0.0)
        nc.sync.dma_start(loss_out[:], z[0:1, 0:1])
        d = sm_pool.tile([1, 8], F32, tag="dbgv")
        nc.vector.memset(d[:], 0.0)
        nc.sync.dma_start(dbg_out[:], d[:])

    # ---------- formation: A = G G^T / n + I ----------
    gpan = pan_pool.tile([P, cw], F32, tag="pan")
    nc.sync.dma_start(gpan[:], git_pan[:])

    zt0 = sm_pool.tile([1, 2], F32, tag="tail")
    nc.vector.memset(zt0[:], 0.0)
    agG, _ = do_allgather(gpan, zt0)         # full G^T, panelized

    # L^T panels: AG early too (full L^T needed for the M pass)
    ltpan = pan_pool.tile([P, cw], F32, tag="pan")
    nc.sync.dma_start(ltpan[:], lti_pan[:])
    zt1 = sm_pool.tile([1, 2], F32, tag="tail")
    nc.vector.memset(zt1[:], 0.0)
    agLT, _ = do_allgather(ltpan, zt1)       # full L^T, panelized

    apan = pan_pool.tile([P, cw], F32, tag="pan")

    def evict_a(m, psum_ap):
        sl = slice(m * pw, (m + 1) * pw)
        nc.scalar.activation(apan[:, sl], psum_ap, ACT.Copy, scale=1.0 / n)
        nc.vector.tensor_add(apan[:, sl], apan[:, sl], ei[:, sl])

    mm_pass(agG[:], gpan, evict_a)

    if debug_stage == "A":
        _dbg_finish(apan)
        return

    # ---------- AG(A); W = A L^T ----------
    zt = sm_pool.tile([1, 2], F32, tag="tail")
    nc.vector.memset(zt[:], 0.0)
    agA, _ = do_allgather(apan, zt)

    wpan = pan_pool.tile([P, cw], F32, tag="pan")

    def evict_plain(dst):
        def fn(m, psum_ap):
            sl = slice(m * pw, (m + 1) * pw)
            nc.scalar.activation(dst[:, sl], psum_ap, ACT.Copy)
        return fn

    mm_pass(agA[:], ltpan, evict_plain(wpan))

    if debug_stage == "W":
        _dbg_finish(wpan)
        return

    # ---------- M = L W ----------
    mpan = pan_pool.tile([P, cw], F32, tag="pan")
    mm_pass(agLT[:], wpan, evict_plain(mpan))
    nc.sync.dma_start(m_store[:], mpan[:])

    if debug_stage == "M":
        _dbg_finish(mpan)
        return

    # persistent chain state
    t_cur = st_pool.tile([P, 1], F32)
    s_acc = st_pool.tile([P, 1], F32)
    ln_lam1 = st_pool.tile([P, 1], F32)
    mu = st_pool.tile([P, 1], F32)
    trMg = st_pool.tile([P, 1], F32)   # global trace of M

    def chain(x0_tile, K, init_t_fn, aux0=None):
        """Squaring chain; returns ln(lam_hat) as a [P,1] tile."""
        nc.vector.memset(s_acc[:], 0.0)
        xpan = x0_tile
        f_rep = fnorm_partial(xpan)
        for k in range(K + 1):
            tail = make_tail(f_rep, aux0 if k == 0 else None)
            if k < K:
                ag_out, totals = do_allgather(xpan, tail)
            else:
                totals = tiny_allgather(tail)
            if k == 0:
                init_t_fn(totals)
            tau = sm_pool.tile([P, 1], F32, tag="tau")
            nc.scalar.activation(tau[:], t_cur[:], ACT.Ln)
            if k < K:
                nc.vector.tensor_add(s_acc[:], s_acc[:], tau[:])
                nc.scalar.mul(s_acc[:], s_acc[:], 2.0)
            inv = sm_pool.tile([P, 1], F32, tag="inv")
            nc.vector.reciprocal(inv[:], t_cur[:])
            inv2 = sm_pool.tile([P, 1], F32, tag="inv2")
            nc.vector.tensor_tensor(out=inv2[:], in0=inv[:], in1=inv[:],
                                    op=ALU.mult)
            # t_next = F_tot / t^2
            nc.vector.tensor_tensor(out=t_cur[:], in0=totals[:, 0:1],
                                    in1=inv2[:], op=ALU.mult)
            if k == K:
                tau2 = sm_pool.tile([P, 1], F32, tag="tau2")
                nc.scalar.activation(tau2[:], t_cur[:], ACT.Ln)
                res = sm_pool.tile([P, 1], F32, tag="chainres")
                nc.vector.tensor_add(res[:], s_acc[:], tau[:])
                nc.vector.tensor_add(res[:], res[:], tau2[:])
                nc.scalar.mul(res[:], res[:], 1.0 / (2 ** K))
                return res
            xnew = pan_pool.tile([P, cw], F32, tag="pan")

            def evict_scaled(m, psum_ap, dst=xnew, sc=inv2):
                sl = slice(m * pw, (m + 1) * pw)
                nc.scalar.activation(dst[:, sl], psum_ap, ACT.Copy,
                                     scale=sc[:])
            mm_pass(ag_out[:], xpan, evict_scaled)
            xpan = xnew
            f_rep = fnorm_partial(xpan)
        raise AssertionError("unreachable")

    # ---------- chain 1: lambda_max of M ----------
    trM_loc = diag_partial(mpan)

    def init_t_chain1(totals):
        nc.vector.tensor_copy(t_cur[:], totals[:, 1:2])
        nc.vector.tensor_copy(trMg[:], totals[:, 1:2])

    if debug_stage == "T0":
        f_rep = fnorm_partial(mpan)
        tail = make_tail(f_rep, trM_loc)
        ag_out, totals = do_allgather(mpan, tail)
        init_t_chain1(totals)
        tau = sm_pool.tile([P, 1], F32, tag="tau")
        nc.scalar.activation(tau[:], t_cur[:], ACT.Ln)
        inv = sm_pool.tile([P, 1], F32, tag="inv")
        nc.vector.reciprocal(inv[:], t_cur[:])
        dbg = sm_pool.tile([1, 8], F32, tag="dbgv")
        nc.vector.memset(dbg[:], 0.0)
        nc.vector.tensor_copy(dbg[:, 0:1], t_cur[0:1, :])
        nc.vector.tensor_copy(dbg[:, 1:2], totals[0:1, 0:1])
        nc.vector.tensor_copy(dbg[:, 2:3], tau[0:1, :])
        nc.vector.tensor_copy(dbg[:, 3:4], inv[0:1, :])
        nc.vector.tensor_copy(dbg[:, 4:5], f_rep[0:1, :])
        nc.vector.tensor_copy(dbg[:, 5:6], trM_loc[0:1, :])
        nc.sync.dma_start(dbg_out[:], dbg[:])
        z = sm_pool.tile([1, 2], F32, tag="tail")
        nc.vector.memset(z[:], 0.0)
        nc.sync.dma_start(loss_out[:], z[0:1, 0:1])
        return

    res1 = chain(mpan, k1, init_t_chain1, aux0=trM_loc)
    nc.vector.tensor_copy(ln_lam1[:], res1[:])

    if debug_stage == "C1":
        dbg = sm_pool.tile([1, 8], F32, tag="dbgv")
        nc.vector.memset(dbg[:], 0.0)
        nc.vector.tensor_copy(dbg[:, 0:1], ln_lam1[0:1, :])
        nc.vector.tensor_copy(dbg[:, 1:2], trMg[0:1, :])
        nc.sync.dma_start(dbg_out[:], dbg[:])
        z = sm_pool.tile([1, 2], F32, tag="tail")
        nc.vector.memset(z[:], 0.0)
        nc.sync.dma_start(loss_out[:], z[0:1, 0:1])
        return

    # ---------- chain 2: mu - lambda_min via B = mu I - M ----------
    nc.scalar.activation(mu[:], ln_lam1[:], ACT.Exp)
    nc.scalar.mul(mu[:], mu[:], MU_FACTOR)

    mpan2 = pan_pool.tile([P, cw], F32, tag="pan")
    nc.sync.dma_start(mpan2[:], m_store[:])
    bpan = pan_pool.tile([P, cw], F32, tag="pan")
    nc.vector.tensor_scalar_mul(bpan[:], ei[:], mu[:])
    nc.vector.tensor_tensor(out=bpan[:], in0=bpan[:], in1=mpan2[:],
                            op=ALU.subtract)

    def init_t_chain2(totals):
        # t0 = n*mu - tr(M)
        nc.scalar.mul(t_cur[:], mu[:], float(n))
        nc.vector.tensor_tensor(out=t_cur[:], in0=t_cur[:],
                                in1=trMg[:], op=ALU.subtract)

    res2 = chain(bpan, k2, init_t_chain2)

    # ---------- final scalar math ----------
    bmax = sm_pool.tile([P, 1], F32, tag="bmax")
    nc.scalar.activation(bmax[:], res2[:], ACT.Exp)
    lam_min = sm_pool.tile([P, 1], F32, tag="lammin")
    nc.vector.tensor_tensor(out=lam_min[:], in0=mu[:], in1=bmax[:],
                            op=ALU.subtract)
    ln_min = sm_pool.tile([P, 1], F32, tag="lnmin")
    nc.scalar.activation(ln_min[:], lam_min[:], ACT.Ln)
    loss = sm_pool.tile([P, 1], F32, tag="lossv")
    nc.vector.tensor_tensor(out=loss[:], in0=ln_lam1[:], in1=ln_min[:],
                            op=ALU.subtract)
    nc.sync.dma_start(loss_out[:], loss[0:1, :])

    dbg = sm_pool.tile([1, 8], F32, tag="dbgv")
    nc.vector.tensor_copy(dbg[:, 0:1], ln_lam1[0:1, :])
    nc.vector.tensor_copy(dbg[:, 1:2], mu[0:1, :])
    nc.vector.tensor_copy(dbg[:, 2:3], bmax[0:1, :])
    nc.vector.tensor_copy(dbg[:, 3:4], lam_min[0:1, :])
    nc.vector.tensor_copy(dbg[:, 4:5], trMg[0:1, :])
    nc.vector.tensor_copy(dbg[:, 5:6], loss[0:1, :])
    nc.sync.dma_start(dbg_out[:], dbg[:])


_NC_CACHE = {}


def _get_nc(n=2048, k1=K1, k2=K2, mm_dt=F32):
    key = (n, k1, k2, str(mm_dt))
    if key not in _NC_CACHE:
        _NC_CACHE[key] = _build_nc(n, k1, k2, mm_dt)
    return _NC_CACHE[key]


def _panelize(mat, i, n):
    """[128, (n//128)*(n//8)] panel of mat[:, i*pw:(i+1)*pw] in SBUF chunk
    layout pan[p, c*pw+j] = mat[c*128+p, i*pw+j]."""
    pw = n // N_CORES
    ch = n // P
    x = mat[:, i * pw:(i + 1) * pw].reshape(ch, P, pw)
    return np.ascontiguousarray(x.transpose(1, 0, 2).reshape(P, ch * pw))


def _pan_full(mat, n):
    """Rank-stacked panelized layout [8*(128+1), cw] matching AG output."""
    pw = n // N_CORES
    ch = n // P
    agr = P + 1
    out = np.zeros((N_CORES * agr, ch * pw), dtype=np.float32)
    for i in range(N_CORES):
        out[i * agr:i * agr + P, :] = _panelize(mat, i, n)
    return out


def _prep_inputs(pred_values, active_scales, A_factor, factor_rows,
                 factor_cols, n):
    G = np.asarray(A_factor, dtype=np.float32)
    GT = np.ascontiguousarray(G.T)
    vals = (np.asarray(pred_values, dtype=np.float32)
            * np.asarray(active_scales, dtype=np.float32))
    LT = np.eye(n, dtype=np.float32)
    np.add.at(LT, (np.asarray(factor_cols), np.asarray(factor_rows)), vals)
    eye = np.eye(n, dtype=np.float32)
    in_maps = []
    for i in range(N_CORES):
        in_maps.append({
            "git_pan": _panelize(GT, i, n),
            "lti_pan": _panelize(LT, i, n),
            "ei_pan": _panelize(eye, i, n),
        })
    return in_maps


_RUNNER_CACHE = {}


def _make_pjrt_runner(nc):
    """Cached jit(shard_map) runner for the axon/PJRT path: avoids the
    per-call retrace that run_bass_via_pjrt pays, so repeat kernel() calls
    cost transfer + execute only."""
    import jax
    from jax.sharding import Mesh, PartitionSpec
    try:
        from jax.experimental.shard_map import shard_map
    except Exception:
        from jax.shard_map import shard_map  # newer jax
    from concourse import bass2jax
    from concourse import mybir as _mybir

    bass2jax.install_neuronx_cc_hook()
    partition_name = (nc.partition_id_tensor.name
                      if nc.partition_id_tensor else None)
    in_names, out_names, out_avals, zero_shapes = [], [], [], []
    for alloc in nc.m.functions[0].allocations:
        if not isinstance(alloc, _mybir.MemoryLocationSet):
            continue
        name = alloc.memorylocations[0].name
        if alloc.kind == "ExternalInput":
            if name != partition_name:
                in_names.append(name)
        elif alloc.kind == "ExternalOutput":
            out_names.append(name)
            shape = tuple(alloc.tensor_shape)
            dtype = _mybir.dt.np(alloc.dtype)
            out_avals.append(jax.core.ShapedArray(shape, dtype))
            zero_shapes.append((shape, dtype))
    n_params = len(in_names)
    all_in_names = list(in_names) + list(out_names)
    if partition_name is not None:
        all_in_names.append(partition_name)
    donate = tuple(range(n_params, n_params + len(out_names)))

    def _body(*args):
        operands = list(args)
        if partition_name is not None:
            operands.append(bass2jax.partition_id_tensor())
        outs = bass2jax._bass_exec_p.bind(
            *operands,
            out_avals=tuple(out_avals),
            in_names=tuple(all_in_names),
            out_names=tuple(out_names),
            lowering_input_output_aliases=(),
            sim_require_finite=True,
            sim_require_nnan=True,
            nc=nc,
        )
        return tuple(outs)

    devices = jax.devices()[:N_CORES]
    mesh = Mesh(np.asarray(devices), ("core",))
    n_args = n_params + len(out_names)
    sharded = jax.jit(
        shard_map(_body, mesh=mesh,
                  in_specs=(PartitionSpec("core"),) * n_args,
                  out_specs=(PartitionSpec("core"),) * len(out_names),
                  check_rep=False),
        donate_argnums=donate, keep_unused=True)

    def run(in_maps):
        concat_in = [
            np.concatenate([np.asarray(in_maps[c][nm]) for c in range(N_CORES)],
                           axis=0)
            for nm in in_names
        ]
        concat_zeros = [
            np.zeros((N_CORES * s[0],) + tuple(s[1:]), dt)
            for (s, dt) in zero_shapes
        ]
        out_arrs = sharded(*concat_in, *concat_zeros)
        res = []
        for c in range(N_CORES):
            res.append({
                nm: np.asarray(out_arrs[i]).reshape(
                    N_CORES, *out_avals[i].shape)[c]
                for i, nm in enumerate(out_names)
            })
        return res

    return run


def _run(nc, in_maps):
    from concourse._compat import axon_active
    if axon_active():
        key = id(nc)
        if key not in _RUNNER_CACHE:
            _RUNNER_CACHE[key] = _make_pjrt_runner(nc)
        return _RUNNER_CACHE[key](in_maps)
    return run_bass_kernel_spmd(
        nc, in_maps, core_ids=list(range(N_CORES))).results


def kernel(pred_values, active_scales, A_factor, factor_rows, factor_cols):
    n = A_factor.shape[0]
    nc = _get_nc(n=n)
    in_maps = _prep_inputs(pred_values, active_scales, A_factor,
                           factor_rows, factor_cols, n)
    results = _run(nc, in_maps)
    out = results[0]["loss"]
    return np.float32(out[0, 0])


if __name__ == "__main__":
    import reference, jax
    cpu = jax.devices("cpu")[0]
    with jax.default_device(cpu):
        inputs = {k: np.asarray(v) for k, v in reference.setup_inputs().items()}
    got = kernel(**inputs)
    print("kernel loss:", got)


# revision 24
# speedup vs baseline: 377.1544x; 53.4469x over previous
"""Trainium2 Bass kernel for nn_CachedConditionNumberLoss.

Computes loss = log(lambda_max) - log(lambda_min) of M = L A L^T where
A = G G^T/n + I  (G = A_factor, n = 2048) and L = I + scatter(pred*scale).

Strategy (8-core SPMD, column-panel sharded):
  - core i owns the column panel X[:, i*PW:(i+1)*PW] of every 2048x2048
    matrix involved; all cross-core exchange is AllGather of panels.
  - device computes A = G G^T/n + I, W = A L^T, M = L W (three panel
    matmul passes), then extremal eigenvalues of M via two repeated-
    squaring chains with trace-ratio estimators:
      chain 1 on M          -> lambda_max
      chain 2 on mu*I - M   -> mu - lambda_min   (mu = 1.001*lambda_max)
    Each squaring step: AllGather panels of X_k (with the Frobenius-norm
    partial embedded in a tail row), then each core computes
    X_{k+1}[:, panel] = (X_k^T X_k)[:, panel] / t_k^2 with PE matmuls
    (lhsT tiles are read straight from the gathered copy; symmetry of X_k
    makes transposes unnecessary).  Trace bookkeeping on device:
      t_{k+1} = ||X_k||_F^2 / t_k^2,  tau_k = ln t_k,
      s_{k+1} = 2 (s_k + tau_k),
      ln(lam_hat) = (s_K + tau_K + tau_{K+1}) / 2^K.
  - host only transposes/slices inputs (incl. assembling L^T from the
    scatter triplets) and reads back the scalar.
"""

import numpy as np

import concourse.tile as tile
from concourse import bacc, mybir
from concourse.bass_utils import run_bass_kernel_spmd

F32 = mybir.dt.float32
ACT = mybir.ActivationFunctionType
ALU = mybir.AluOpType
P = 128
N_CORES = 8

# squaring-chain lengths (validated in fp32 simulation: loss relerr ~2e-7)
K1 = 11
K2 = 14
MU_FACTOR = 1.001


def _build_nc(n=2048, k1=K1, k2=K2, mm_dt=F32, debug_stage=None, repeats=1):
    ch = n // P           # 128-row chunks per matrix (16)
    pw = n // N_CORES     # panel width per core (256)
    cw = ch * pw          # panel free size in SBUF layout (4096)
    agr = P + 1           # rows per rank in AG buffers (tail row at P)
    cpp = pw // P         # column chunks per panel (2)

    nc = bacc.Bacc(None, target_bir_lowering=False)

    git_pan = nc.dram_tensor("git_pan", [P, cw], F32, kind="ExternalInput")
    lti_pan = nc.dram_tensor("lti_pan", [P, cw], F32, kind="ExternalInput")
    ei_pan = nc.dram_tensor("ei_pan", [P, cw], F32, kind="ExternalInput")

    loss_out = nc.dram_tensor("loss", [1, 1], F32, kind="ExternalOutput")
    dbg_out = nc.dram_tensor("dbg", [1, 8], F32, kind="ExternalOutput")

    m_store = nc.dram_tensor("m_store", [P, cw], F32, kind="Internal")
    pan_out = (nc.dram_tensor("pan_out", [P, cw], F32, kind="ExternalOutput")
               if debug_stage in ("A", "W", "M") else None)

    with tile.TileContext(nc) as tc:
        with (
            tc.tile_pool(name="xf", bufs=7) as xf_pool,
            tc.tile_pool(name="pan", bufs=3) as pan_pool,
            tc.tile_pool(name="eip", bufs=1) as ei_pool,
            tc.tile_pool(name="small", bufs=6) as sm_pool,
            tc.tile_pool(name="state", bufs=1) as st_pool,
            tc.tile_pool(name="psum", bufs=6, space="PSUM") as ps_pool,
            tc.tile_pool(name="psr", bufs=2, space="PSUM") as psr_pool,
            tc.tile_pool(name="dram", bufs=2, space="DRAM") as dram_pool,
        ):
            for _rep in range(repeats):
                _trace_program(
                    nc, n, k1, k2, mm_dt, debug_stage,
                    ch, pw, cw, agr, cpp,
                    git_pan, lti_pan, ei_pan,
                    loss_out, dbg_out, m_store, pan_out,
                    xf_pool, pan_pool, ei_pool, sm_pool, st_pool,
                    ps_pool, psr_pool, dram_pool,
                )

    nc.compile()
    return nc


def _trace_program(nc, n, k1, k2, mm_dt, debug_stage,
                   ch, pw, cw, agr, cpp,
                   git_pan, lti_pan, ei_pan,
                   loss_out, dbg_out, m_store, pan_out,
                   xf_pool, pan_pool, ei_pool, sm_pool, st_pool,
                   ps_pool, psr_pool, dram_pool):
    ones = st_pool.tile([P, P], F32)
    nc.vector.memset(ones[:], 1.0)

    ei = ei_pool.tile([P, cw], F32, tag="ei")
    nc.sync.dma_start(ei[:], ei_pan[:])

    # ---------- helpers ----------
    def part_reduce(vec_ap, width=1):
        """[p, width] -> [P, width] replicated column sums."""
        red = psr_pool.tile([P, 2], F32, space="PSUM", tag="red")
        p_sz = vec_ap.shape[0]
        nc.tensor.matmul(red[:, 0:width], lhsT=ones[:p_sz, :],
                         rhs=vec_ap, start=True, stop=True)
        out = sm_pool.tile([P, width], F32, tag="pred")
        nc.vector.tensor_copy(out[:], red[:, 0:width])
        return out

    def fnorm_partial(pan_tile):
        """sum of squares of a [P, cw] panel -> [P,1] replicated."""
        acc = sm_pool.tile([P, ch], F32, tag="facc")
        for c in range(ch):
            tmp = sm_pool.tile([P, pw], F32, tag="sqtmp")
            nc.scalar.activation(tmp[:], pan_tile[:, c * pw:(c + 1) * pw],
                                 ACT.Square, accum_out=acc[:, c:c + 1])
        accs = sm_pool.tile([P, 1], F32, tag="faccs")
        nc.vector.reduce_sum(accs[:], acc[:], axis=mybir.AxisListType.X)
        return part_reduce(accs[:])

    def diag_partial(pan_tile):
        """sum of (panel .* ei) -> [P,1] replicated local trace partial."""
        acc = sm_pool.tile([P, ch], F32, tag="facc")
        for c in range(ch):
            sl = slice(c * pw, (c + 1) * pw)
            tmp = sm_pool.tile([P, pw], F32, tag="sqtmp")
            nc.vector.tensor_tensor(out=tmp[:], in0=pan_tile[:, sl],
                                    in1=ei[:, sl], op=ALU.mult)
            nc.vector.reduce_sum(acc[:, c:c + 1], tmp[:],
                                 axis=mybir.AxisListType.X)
        accs = sm_pool.tile([P, 1], F32, tag="faccs")
        nc.vector.reduce_sum(accs[:], acc[:], axis=mybir.AxisListType.X)
        return part_reduce(accs[:])

    f32r = (mm_dt == mybir.dt.float32r)

    def _mmcast(ap):
        return ap.bitcast(mybir.dt.float32r) if f32r else ap

    def mm_pass(src_dram, rhs_tile, evict_fn):
        """out[:, panel] = X^T @ rhs_panel, X stored panelized in src_dram."""
        tiles = []
        for r in range(N_CORES):
            t = xf_pool.tile([P, cw], F32, tag="xf")
            nc.sync.dma_start(t[:], src_dram[r * agr:r * agr + P, :])
            tiles.append(t)
        for m in range(ch):
            acc = ps_pool.tile([P, pw], F32, space="PSUM", tag="mm")
            t = tiles[m // cpp]
            base = (m % cpp) * P
            for k in range(ch):
                nc.tensor.matmul(
                    acc[:],
                    lhsT=_mmcast(t[:, k * pw + base:k * pw + base + P]),
                    rhs=_mmcast(rhs_tile[:, k * pw:(k + 1) * pw]),
                    start=(k == 0), stop=(k == ch - 1),
                )
            evict_fn(m, acc[:])

    def do_allgather(pan_tile, tail_tile):
        """AG panels + 2-value tail; returns (ag_out, totals[P,2])."""
        ag_in = dram_pool.tile([agr, cw], F32, tag="agin")
        ag_out = dram_pool.tile([N_CORES * agr, cw], F32, tag="agout",
                                addr_space="Shared")
        nc.sync.dma_start(ag_in[0:P, :], pan_tile[:])
        nc.sync.dma_start(ag_in[P:P + 1, 0:2], tail_tile[0:1, 0:2])
        nc.gpsimd.collective_compute(
            "AllGather", ALU.bypass,
            ins=[ag_in[:]], outs=[ag_out[:]],
            replica_groups=[list(range(N_CORES))],
        )
        tails8 = sm_pool.tile([N_CORES, 2], F32, tag="tails8")
        nc.sync.dma_start(
            tails8[:],
            ag_out.rearrange("(r p) c -> r p c", p=agr)[:, P:P + 1, 0:2])
        totals = part_reduce(tails8[:], width=2)
        return ag_out, totals

    def tiny_allgather(tail_tile):
        agt_in = dram_pool.tile([1, 16], F32, tag="agtin")
        agt_out = dram_pool.tile([N_CORES, 16], F32, tag="agtout",
                                 addr_space="Shared")
        pad = sm_pool.tile([1, 16], F32, tag="tailpad")
        nc.vector.memset(pad[:], 0.0)
        nc.vector.tensor_copy(pad[:, 0:2], tail_tile[0:1, 0:2])
        nc.sync.dma_start(agt_in[:], pad[:])
        nc.gpsimd.collective_compute(
            "AllGather", ALU.bypass,
            ins=[agt_in[:]], outs=[agt_out[:]],
            replica_groups=[list(range(N_CORES))],
        )
        t8 = sm_pool.tile([N_CORES, 2], F32, tag="tails8")
        nc.sync.dma_start(t8[:], agt_out[:, 0:2])
        return part_reduce(t8[:], width=2)

    def make_tail(f_rep, aux_rep=None):
        t = sm_pool.tile([1, 2], F32, tag="tail")
        nc.vector.tensor_copy(t[:, 0:1], f_rep[0:1, :])
        if aux_rep is not None:
            nc.vector.tensor_copy(t[:, 1:2], aux_rep[0:1, :])
        else:
            nc.vector.memset(t[:, 1:2], 0.0)
        return t

    def _dbg_finish(tile_):
        nc.sync.dma_start(pan_out[:], tile_[:])
        z = sm_pool.tile([1, 2], F32, tag="tail")
        nc.vector.memset(z[:], 0.0)
        nc.sync.dma_start(loss_out[:], z[0:1, 0:1])
        d = sm_pool.tile([1, 8], F32, tag="dbgv")
        nc.vector.memset(d[:], 0.0)
        nc.sync.dma_start(dbg_out[:], d[:])

    # ---------- formation: A = G G^T / n + I ----------
    gpan = pan_pool.tile([P, cw], F32, tag="pan")
    nc.sync.dma_start(gpan[:], git_pan[:])

    zt0 = sm_pool.tile([1, 2], F32, tag="tail")
    nc.vector.memset(zt0[:], 0.0)
    agG, _ = do_allgather(gpan, zt0)         # full G^T, panelized

    # L^T panels: AG early too (full L^T needed for the M pass)
    ltpan = pan_pool.tile([P, cw], F32, tag="pan")
    nc.sync.dma_start(ltpan[:], lti_pan[:])
    zt1 = sm_pool.tile([1, 2], F32, tag="tail")
    nc.vector.memset(zt1[:], 0.0)
    agLT, _ = do_allgather(ltpan, zt1)       # full L^T, panelized

    apan = pan_pool.tile([P, cw], F32, tag="pan")

    def evict_a(m, psum_ap):
        sl = slice(m * pw, (m + 1) * pw)
        nc.scalar.activation(apan[:, sl], psum_ap, ACT.Copy, scale=1.0 / n)
        nc.vector.tensor_add(apan[:, sl], apan[:, sl], ei[:, sl])

    mm_pass(agG[:], gpan, evict_a)

    if debug_stage == "A":
        _dbg_finish(apan)
        return

    # ---------- AG(A); W = A L^T ----------
    zt = sm_pool.tile([1, 2], F32, tag="tail")
    nc.vector.memset(zt[:], 0.0)
    agA, _ = do_allgather(apan, zt)

    wpan = pan_pool.tile([P, cw], F32, tag="pan")

    def evict_plain(dst):
        def fn(m, psum_ap):
            sl = slice(m * pw, (m + 1) * pw)
            nc.scalar.activation(dst[:, sl], psum_ap, ACT.Copy)
        return fn

    mm_pass(agA[:], ltpan, evict_plain(wpan))

    if debug_stage == "W":
        _dbg_finish(wpan)
        return

    # ---------- M = L W ----------
    mpan = pan_pool.tile([P, cw], F32, tag="pan")
    mm_pass(agLT[:], wpan, evict_plain(mpan))
    nc.sync.dma_start(m_store[:], mpan[:])

    if debug_stage == "M":
        _dbg_finish(mpan)
        return

    # persistent chain state
    t_cur = st_pool.tile([P, 1], F32)
    s_acc = st_pool.tile([P, 1], F32)
    ln_lam1 = st_pool.tile([P, 1], F32)
    mu = st_pool.tile([P, 1], F32)
    trMg = st_pool.tile([P, 1], F32)   # global trace of M

    def chain(x0_tile, K, init_t_fn, aux0=None):
        """Squaring chain; returns ln(lam_hat) as a [P,1] tile."""
        nc.vector.memset(s_acc[:], 0.0)
        xpan = x0_tile
        f_rep = fnorm_partial(xpan)
        for k in range(K + 1):
            tail = make_tail(f_rep, aux0 if k == 0 else None)
            if k < K:
                ag_out, totals = do_allgather(xpan, tail)
            else:
                totals = tiny_allgather(tail)
            if k == 0:
                init_t_fn(totals)
            tau = sm_pool.tile([P, 1], F32, tag="tau")
            nc.scalar.activation(tau[:], t_cur[:], ACT.Ln)
            if k < K:
                nc.vector.tensor_add(s_acc[:], s_acc[:], tau[:])
                nc.scalar.mul(s_acc[:], s_acc[:], 2.0)
            inv = sm_pool.tile([P, 1], F32, tag="inv")
            nc.vector.reciprocal(inv[:], t_cur[:])
            inv2 = sm_pool.tile([P, 1], F32, tag="inv2")
            nc.vector.tensor_tensor(out=inv2[:], in0=inv[:], in1=inv[:],
                                    op=ALU.mult)
            # t_next = F_tot / t^2
            nc.vector.tensor_tensor(out=t_cur[:], in0=totals[:, 0:1],
                                    in1=inv2[:], op=ALU.mult)
            if k == K:
                tau2 = sm_pool.tile([P, 1], F32, tag="tau2")
                nc.scalar.activation(tau2[:], t_cur[:], ACT.Ln)
                res = sm_pool.tile([P, 1], F32, tag="chainres")
                nc.vector.tensor_add(res[:], s_acc[:], tau[:])
                nc.vector.tensor_add(res[:], res[:], tau2[:])
                nc.scalar.mul(res[:], res[:], 1.0 / (2 ** K))
                return res
            xnew = pan_pool.tile([P, cw], F32, tag="pan")

            def evict_scaled(m, psum_ap, dst=xnew, sc=inv2):
                sl = slice(m * pw, (m + 1) * pw)
                nc.scalar.activation(dst[:, sl], psum_ap, ACT.Copy,
                                     scale=sc[:])
            mm_pass(ag_out[:], xpan, evict_scaled)
            xpan = xnew
            f_rep = fnorm_partial(xpan)
        raise AssertionError("unreachable")

    # ---------- chain 1: lambda_max of M ----------
    trM_loc = diag_partial(mpan)

    def init_t_chain1(totals):
        nc.vector.tensor_copy(t_cur[:], totals[:, 1:2])
        nc.vector.tensor_copy(trMg[:], totals[:, 1:2])

    if debug_stage == "T0":
        f_rep = fnorm_partial(mpan)
        tail = make_tail(f_rep, trM_loc)
        ag_out, totals = do_allgather(mpan, tail)
        init_t_chain1(totals)
        tau = sm_pool.tile([P, 1], F32, tag="tau")
        nc.scalar.activation(tau[:], t_cur[:], ACT.Ln)
        inv = sm_pool.tile([P, 1], F32, tag="inv")
        nc.vector.reciprocal(inv[:], t_cur[:])
        dbg = sm_pool.tile([1, 8], F32, tag="dbgv")
        nc.vector.memset(dbg[:], 0.0)
        nc.vector.tensor_copy(dbg[:, 0:1], t_cur[0:1, :])
        nc.vector.tensor_copy(dbg[:, 1:2], totals[0:1, 0:1])
        nc.vector.tensor_copy(dbg[:, 2:3], tau[0:1, :])
        nc.vector.tensor_copy(dbg[:, 3:4], inv[0:1, :])
        nc.vector.tensor_copy(dbg[:, 4:5], f_rep[0:1, :])
        nc.vector.tensor_copy(dbg[:, 5:6], trM_loc[0:1, :])
        nc.sync.dma_start(dbg_out[:], dbg[:])
        z = sm_pool.tile([1, 2], F32, tag="tail")
        nc.vector.memset(z[:], 0.0)
        nc.sync.dma_start(loss_out[:], z[0:1, 0:1])
        return

    res1 = chain(mpan, k1, init_t_chain1, aux0=trM_loc)
    nc.vector.tensor_copy(ln_lam1[:], res1[:])

    if debug_stage == "C1":
        dbg = sm_pool.tile([1, 8], F32, tag="dbgv")
        nc.vector.memset(dbg[:], 0.0)
        nc.vector.tensor_copy(dbg[:, 0:1], ln_lam1[0:1, :])
        nc.vector.tensor_copy(dbg[:, 1:2], trMg[0:1, :])
        nc.sync.dma_start(dbg_out[:], dbg[:])
        z = sm_pool.tile([1, 2], F32, tag="tail")
        nc.vector.memset(z[:], 0.0)
        nc.sync.dma_start(loss_out[:], z[0:1, 0:1])
        return

    # ---------- chain 2: mu - lambda_min via B = mu I - M ----------
    nc.scalar.activation(mu[:], ln_lam1[:], ACT.Exp)
    nc.scalar.mul(mu[:], mu[:], MU_FACTOR)

    mpan2 = pan_pool.tile([P, cw], F32, tag="pan")
    nc.sync.dma_start(mpan2[:], m_store[:])
    bpan = pan_pool.tile([P, cw], F32, tag="pan")
    nc.vector.tensor_scalar_mul(bpan[:], ei[:], mu[:])
    nc.vector.tensor_tensor(out=bpan[:], in0=bpan[:], in1=mpan2[:],
                            op=ALU.subtract)

    def init_t_chain2(totals):
        # t0 = n*mu - tr(M)
        nc.scalar.mul(t_cur[:], mu[:], float(n))
        nc.vector.tensor_tensor(out=t_cur[:], in0=t_cur[:],
                                in1=trMg[:], op=ALU.subtract)

    res2 = chain(bpan, k2, init_t_chain2)

    # ---------- final scalar math ----------
    bmax = sm_pool.tile([P, 1], F32, tag="bmax")
    nc.scalar.activation(bmax[:], res2[:], ACT.Exp)
    lam_min = sm_pool.tile([P, 1], F32, tag="lammin")
    nc.vector.tensor_tensor(out=lam_min[:], in0=mu[:], in1=bmax[:],
                            op=ALU.subtract)
    ln_min = sm_pool.tile([P, 1], F32, tag="lnmin")
    nc.scalar.activation(ln_min[:], lam_min[:], ACT.Ln)
    loss = sm_pool.tile([P, 1], F32, tag="lossv")
    nc.vector.tensor_tensor(out=loss[:], in0=ln_lam1[:], in1=ln_min[:],
                            op=ALU.subtract)
    nc.sync.dma_start(loss_out[:], loss[0:1, :])

    dbg = sm_pool.tile([1, 8], F32, tag="dbgv")
    nc.vector.tensor_copy(dbg[:, 0:1], ln_lam1[0:1, :])
    nc.vector.tensor_copy(dbg[:, 1:2], mu[0:1, :])
    nc.vector.tensor_copy(dbg[:, 2:3], bmax[0:1, :])
    nc.vector.tensor_copy(dbg[:, 3:4], lam_min[0:1, :])
    nc.vector.tensor_copy(dbg[:, 4:5], trMg[0:1, :])
    nc.vector.tensor_copy(dbg[:, 5:6], loss[0:1, :])
    nc.sync.dma_start(dbg_out[:], dbg[:])


_NC_CACHE = {}


def _get_nc(n=2048, k1=K1, k2=K2, mm_dt=F32):
    key = (n, k1, k2, str(mm_dt))
    if key not in _NC_CACHE:
        _NC_CACHE[key] = _build_nc(n, k1, k2, mm_dt)
    return _NC_CACHE[key]


def _panelize(mat, i, n):
    """[128, (n//128)*(n//8)] panel of mat[:, i*pw:(i+1)*pw] in SBUF chunk
    layout pan[p, c*pw+j] = mat[c*128+p, i*pw+j]."""
    pw = n // N_CORES
    ch = n // P
    x = mat[:, i * pw:(i + 1) * pw].reshape(ch, P, pw)
    return np.ascontiguousarray(x.transpose(1, 0, 2).reshape(P, ch * pw))


def _prep_inputs(pred_values, active_scales, A_factor, factor_rows,
                 factor_cols, n):
    G = np.asarray(A_factor, dtype=np.float32)
    GT = np.ascontiguousarray(G.T)
    vals = (np.asarray(pred_values, dtype=np.float32)
            * np.asarray(active_scales, dtype=np.float32))
    LT = np.eye(n, dtype=np.float32)
    np.add.at(LT, (np.asarray(factor_cols), np.asarray(factor_rows)), vals)
    eye = np.eye(n, dtype=np.float32)
    in_maps = []
    for i in range(N_CORES):
        in_maps.append({
            "git_pan": _panelize(GT, i, n),
            "lti_pan": _panelize(LT, i, n),
            "ei_pan": _panelize(eye, i, n),
        })
    return in_maps


_RUNNER_CACHE = {}


def _make_pjrt_runner(nc):
    """Cached jit(shard_map) runner for the axon/PJRT path: avoids the
    per-call retrace that run_bass_via_pjrt pays, so repeat kernel() calls
    cost transfer + execute only."""
    import jax
    from jax.sharding import Mesh, PartitionSpec
    try:
        from jax.experimental.shard_map import shard_map
    except Exception:
        from jax.shard_map import shard_map  # newer jax
    from concourse import bass2jax
    from concourse import mybir as _mybir

    bass2jax.install_neuronx_cc_hook()
    partition_name = (nc.partition_id_tensor.name
                      if nc.partition_id_tensor else None)
    in_names, out_names, out_avals, zero_shapes = [], [], [], []
    for alloc in nc.m.functions[0].allocations:
        if not isinstance(alloc, _mybir.MemoryLocationSet):
            continue
        name = alloc.memorylocations[0].name
        if alloc.kind == "ExternalInput":
            if name != partition_name:
                in_names.append(name)
        elif alloc.kind == "ExternalOutput":
            out_names.append(name)
            shape = tuple(alloc.tensor_shape)
            dtype = _mybir.dt.np(alloc.dtype)
            out_avals.append(jax.core.ShapedArray(shape, dtype))
            zero_shapes.append((shape, dtype))
    n_params = len(in_names)
    all_in_names = list(in_names) + list(out_names)
    if partition_name is not None:
        all_in_names.append(partition_name)
    donate = tuple(range(n_params, n_params + len(out_names)))

    def _body(*args):
        operands = list(args)
        if partition_name is not None:
            operands.append(bass2jax.partition_id_tensor())
        outs = bass2jax._bass_exec_p.bind(
            *operands,
            out_avals=tuple(out_avals),
            in_names=tuple(all_in_names),
            out_names=tuple(out_names),
            lowering_input_output_aliases=(),
            sim_require_finite=True,
            sim_require_nnan=True,
            nc=nc,
        )
        return tuple(outs)

    devices = jax.devices()[:N_CORES]
    mesh = Mesh(np.asarray(devices), ("core",))
    n_args = n_params + len(out_names)
    sharded = jax.jit(
        shard_map(_body, mesh=mesh,
                  in_specs=(PartitionSpec("core"),) * n_args,
                  out_specs=(PartitionSpec("core"),) * len(out_names),
                  check_rep=False),
        donate_argnums=donate, keep_unused=True)

    def run(in_maps):
        concat_in = [
            np.concatenate([np.asarray(in_maps[c][nm]) for c in range(N_CORES)],
                           axis=0)
            for nm in in_names
        ]
        concat_zeros = [
            np.zeros((N_CORES * s[0],) + tuple(s[1:]), dt)
            for (s, dt) in zero_shapes
        ]
        out_arrs = sharded(*concat_in, *concat_zeros)
        res = []
        for c in range(N_CORES):
            res.append({
                nm: np.asarray(out_arrs[i]).reshape(
                    N_CORES, *out_avals[i].shape)[c]
                for i, nm in enumerate(out_names)
            })
        return res

    return run


def _run(nc, in_maps):
    from concourse._compat import axon_active
    if axon_active():
        key = id(nc)
        if key not in _RUNNER_CACHE:
            _RUNNER_CACHE[key] = _make_pjrt_runner(nc)
        return _RUNNER_CACHE[key](in_maps)
    return run_bass_kernel_spmd(
        nc, in_maps, core_ids=list(range(N_CORES))).results


def kernel(pred_values, active_scales, A_factor, factor_rows, factor_cols):
    n = A_factor.shape[0]
    nc = _get_nc(n=n)
    in_maps = _prep_inputs(pred_values, active_scales, A_factor,
                           factor_rows, factor_cols, n)
    results = _run(nc, in_maps)
    out = results[0]["loss"]
    return np.float32(out[0, 0])


if __name__ == "__main__":
    import reference, jax
    cpu = jax.devices("cpu")[0]
    with jax.default_device(cpu):
        inputs = {k: np.asarray(v) for k, v in reference.setup_inputs().items()}
    got = kernel(**inputs)
    print("kernel loss:", got)


# revision 25
# speedup vs baseline: 440.6877x; 1.1685x over previous
"""Trainium2 Bass kernel for nn_CachedConditionNumberLoss.

Computes loss = log(lambda_max) - log(lambda_min) of M = L A L^T where
A = G G^T/n + I  (G = A_factor, n = 2048) and L = I + scatter(pred*scale).

Strategy (8-core SPMD, column-panel sharded):
  - core i owns the column panel X[:, i*PW:(i+1)*PW] of every 2048x2048
    matrix involved; all cross-core exchange is AllGather of panels.
  - device computes A = G G^T/n + I, W = A L^T, M = L W (three panel
    matmul passes), then extremal eigenvalues of M via two repeated-
    squaring chains with trace-ratio estimators:
      chain 1 on M          -> lambda_max
      chain 2 on mu*I - M   -> mu - lambda_min   (mu = 1.001*lambda_max)
    Each squaring step: AllGather panels of X_k (with the Frobenius-norm
    partial embedded in a tail row), then each core computes
    X_{k+1}[:, panel] = (X_k^T X_k)[:, panel] / t_k^2 with PE matmuls
    (lhsT tiles are read straight from the gathered copy; symmetry of X_k
    makes transposes unnecessary).  Trace bookkeeping on device:
      t_{k+1} = ||X_k||_F^2 / t_k^2,  tau_k = ln t_k,
      s_{k+1} = 2 (s_k + tau_k),
      ln(lam_hat) = (s_K + tau_K + tau_{K+1}) / 2^K.
  - host only transposes/slices inputs (incl. assembling L^T from the
    scatter triplets) and reads back the scalar.
"""

import numpy as np

import concourse.tile as tile
from concourse import bacc, mybir
from concourse.bass_utils import run_bass_kernel_spmd

F32 = mybir.dt.float32
ACT = mybir.ActivationFunctionType
ALU = mybir.AluOpType
P = 128
N_CORES = 8

# squaring-chain lengths (validated in fp32 simulation: loss relerr ~2e-7)
K1 = 11
K2 = 14
MU_FACTOR = 1.001


def _build_nc(n=2048, k1=K1, k2=K2, mm_dt=F32, debug_stage=None, repeats=1):
    ch = n // P           # 128-row chunks per matrix (16)
    pw = n // N_CORES     # panel width per core (256)
    cw = ch * pw          # panel free size in SBUF layout (4096)
    agr = P + 1           # rows per rank in AG buffers (tail row at P)
    cpp = pw // P         # column chunks per panel (2)

    nc = bacc.Bacc(None, target_bir_lowering=False)

    git_pan = nc.dram_tensor("git_pan", [P, cw], F32, kind="ExternalInput")
    lti_pan = nc.dram_tensor("lti_pan", [P, cw], F32, kind="ExternalInput")
    ei_pan = nc.dram_tensor("ei_pan", [P, cw], F32, kind="ExternalInput")

    loss_out = nc.dram_tensor("loss", [1, 1], F32, kind="ExternalOutput")
    dbg_out = nc.dram_tensor("dbg", [1, 8], F32, kind="ExternalOutput")

    pan_out = (nc.dram_tensor("pan_out", [P, cw], F32, kind="ExternalOutput")
               if debug_stage in ("A", "W", "M") else None)

    with tile.TileContext(nc) as tc:
        with (
            tc.tile_pool(name="xf", bufs=7) as xf_pool,
            tc.tile_pool(name="pan", bufs=3) as pan_pool,
            tc.tile_pool(name="eip", bufs=1) as ei_pool,
            tc.tile_pool(name="small", bufs=6) as sm_pool,
            tc.tile_pool(name="state", bufs=1) as st_pool,
            tc.tile_pool(name="psum", bufs=6, space="PSUM") as ps_pool,
            tc.tile_pool(name="psr", bufs=2, space="PSUM") as psr_pool,
            tc.tile_pool(name="dram", bufs=2, space="DRAM") as dram_pool,
        ):
            for _rep in range(repeats):
                _trace_program(
                    nc, n, k1, k2, mm_dt, debug_stage,
                    ch, pw, cw, agr, cpp,
                    git_pan, lti_pan, ei_pan,
                    loss_out, dbg_out, pan_out,
                    xf_pool, pan_pool, ei_pool, sm_pool, st_pool,
                    ps_pool, psr_pool, dram_pool,
                )

    nc.compile()
    return nc


def _trace_program(nc, n, k1, k2, mm_dt, debug_stage,
                   ch, pw, cw, agr, cpp,
                   git_pan, lti_pan, ei_pan,
                   loss_out, dbg_out, pan_out,
                   xf_pool, pan_pool, ei_pool, sm_pool, st_pool,
                   ps_pool, psr_pool, dram_pool):
    ones = st_pool.tile([P, P], F32)
    nc.vector.memset(ones[:], 1.0)

    ei = ei_pool.tile([P, cw], F32, tag="ei")
    nc.sync.dma_start(ei[:], ei_pan[:])

    # ---------- helpers ----------
    def part_reduce(vec_ap, width=1):
        """[p, width] -> [P, width] replicated column sums."""
        red = psr_pool.tile([P, 2], F32, space="PSUM", tag="red")
        p_sz = vec_ap.shape[0]
        nc.tensor.matmul(red[:, 0:width], lhsT=ones[:p_sz, :],
                         rhs=vec_ap, start=True, stop=True)
        out = sm_pool.tile([P, width], F32, tag="pred")
        nc.vector.tensor_copy(out[:], red[:, 0:width])
        return out

    def fnorm_partial(pan_tile):
        """sum of squares of a [P, cw] panel -> [P,1] replicated."""
        acc = sm_pool.tile([P, ch], F32, tag="facc")
        for c in range(ch):
            tmp = sm_pool.tile([P, pw], F32, tag="sqtmp")
            nc.scalar.activation(tmp[:], pan_tile[:, c * pw:(c + 1) * pw],
                                 ACT.Square, accum_out=acc[:, c:c + 1])
        accs = sm_pool.tile([P, 1], F32, tag="faccs")
        nc.vector.reduce_sum(accs[:], acc[:], axis=mybir.AxisListType.X)
        return part_reduce(accs[:])

    def diag_partial(pan_tile):
        """sum of (panel .* ei) -> [P,1] replicated local trace partial."""
        acc = sm_pool.tile([P, ch], F32, tag="facc")
        for c in range(ch):
            sl = slice(c * pw, (c + 1) * pw)
            tmp = sm_pool.tile([P, pw], F32, tag="sqtmp")
            nc.vector.tensor_tensor(out=tmp[:], in0=pan_tile[:, sl],
                                    in1=ei[:, sl], op=ALU.mult)
            nc.vector.reduce_sum(acc[:, c:c + 1], tmp[:],
                                 axis=mybir.AxisListType.X)
        accs = sm_pool.tile([P, 1], F32, tag="faccs")
        nc.vector.reduce_sum(accs[:], acc[:], axis=mybir.AxisListType.X)
        return part_reduce(accs[:])

    f32r = (mm_dt == mybir.dt.float32r)

    def _mmcast(ap):
        return ap.bitcast(mybir.dt.float32r) if f32r else ap

    def mm_pass(src_dram, rhs_tile, evict_fn):
        """out[:, panel] = X^T @ rhs_panel, X stored panelized in src_dram."""
        tiles = []
        for r in range(N_CORES):
            t = xf_pool.tile([P, cw], F32, tag="xf")
            nc.sync.dma_start(t[:], src_dram[r * agr:r * agr + P, :])
            tiles.append(t)
        for m in range(ch):
            acc = ps_pool.tile([P, pw], F32, space="PSUM", tag="mm")
            t = tiles[m // cpp]
            base = (m % cpp) * P
            for k in range(ch):
                nc.tensor.matmul(
                    acc[:],
                    lhsT=_mmcast(t[:, k * pw + base:k * pw + base + P]),
                    rhs=_mmcast(rhs_tile[:, k * pw:(k + 1) * pw]),
                    start=(k == 0), stop=(k == ch - 1),
                )
            evict_fn(m, acc[:])

    def do_allgather(pan_tile, tail_tile):
        """AG panels + 2-value tail; returns (ag_out, totals[P,2])."""
        ag_in = dram_pool.tile([agr, cw], F32, tag="agin")
        ag_out = dram_pool.tile([N_CORES * agr, cw], F32, tag="agout",
                                addr_space="Shared")
        nc.sync.dma_start(ag_in[0:P, :], pan_tile[:])
        nc.sync.dma_start(ag_in[P:P + 1, 0:2], tail_tile[0:1, 0:2])
        nc.gpsimd.collective_compute(
            "AllGather", ALU.bypass,
            ins=[ag_in[:]], outs=[ag_out[:]],
            replica_groups=[list(range(N_CORES))],
        )
        tails8 = sm_pool.tile([N_CORES, 2], F32, tag="tails8")
        nc.sync.dma_start(
            tails8[:],
            ag_out.rearrange("(r p) c -> r p c", p=agr)[:, P:P + 1, 0:2])
        totals = part_reduce(tails8[:], width=2)
        return ag_out, totals

    def tiny_allgather(tail_tile):
        agt_in = dram_pool.tile([1, 16], F32, tag="agtin")
        agt_out = dram_pool.tile([N_CORES, 16], F32, tag="agtout",
                                 addr_space="Shared")
        pad = sm_pool.tile([1, 16], F32, tag="tailpad")
        nc.vector.memset(pad[:], 0.0)
        nc.vector.tensor_copy(pad[:, 0:2], tail_tile[0:1, 0:2])
        nc.sync.dma_start(agt_in[:], pad[:])
        nc.gpsimd.collective_compute(
            "AllGather", ALU.bypass,
            ins=[agt_in[:]], outs=[agt_out[:]],
            replica_groups=[list(range(N_CORES))],
        )
        t8 = sm_pool.tile([N_CORES, 2], F32, tag="tails8")
        nc.sync.dma_start(t8[:], agt_out[:, 0:2])
        return part_reduce(t8[:], width=2)

    def make_tail(f_rep, aux_rep=None):
        t = sm_pool.tile([1, 2], F32, tag="tail")
        nc.vector.tensor_copy(t[:, 0:1], f_rep[0:1, :])
        if aux_rep is not None:
            nc.vector.tensor_copy(t[:, 1:2], aux_rep[0:1, :])
        else:
            nc.vector.memset(t[:, 1:2], 0.0)
        return t

    def _dbg_finish(tile_):
        nc.sync.dma_start(pan_out[:], tile_[:])
        z = sm_pool.tile([1, 2], F32, tag="tail")
        nc.vector.memset(z[:], 0.0)
        nc.sync.dma_start(loss_out[:], z[0:1, 0:1])
        d = sm_pool.tile([1, 8], F32, tag="dbgv")
        nc.vector.memset(d[:], 0.0)
        nc.sync.dma_start(dbg_out[:], d[:])

    # ---------- formation: A = G G^T / n + I ----------
    gpan = pan_pool.tile([P, cw], F32, tag="pan")
    nc.sync.dma_start(gpan[:], git_pan[:])

    zt0 = sm_pool.tile([1, 2], F32, tag="tail")
    nc.vector.memset(zt0[:], 0.0)
    agG, _ = do_allgather(gpan, zt0)         # full G^T, panelized

    # L^T panels: AG early too (full L^T needed for the M pass)
    ltpan = pan_pool.tile([P, cw], F32, tag="pan")
    nc.sync.dma_start(ltpan[:], lti_pan[:])
    zt1 = sm_pool.tile([1, 2], F32, tag="tail")
    nc.vector.memset(zt1[:], 0.0)
    agLT, _ = do_allgather(ltpan, zt1)       # full L^T, panelized

    apan = pan_pool.tile([P, cw], F32, tag="pan")

    def evict_a(m, psum_ap):
        sl = slice(m * pw, (m + 1) * pw)
        nc.scalar.activation(apan[:, sl], psum_ap, ACT.Copy, scale=1.0 / n)
        nc.vector.tensor_add(apan[:, sl], apan[:, sl], ei[:, sl])

    mm_pass(agG[:], gpan, evict_a)

    if debug_stage == "A":
        _dbg_finish(apan)
        return

    # ---------- AG(A); W = A L^T ----------
    zt = sm_pool.tile([1, 2], F32, tag="tail")
    nc.vector.memset(zt[:], 0.0)
    agA, _ = do_allgather(apan, zt)

    wpan = pan_pool.tile([P, cw], F32, tag="pan")

    def evict_plain(dst):
        def fn(m, psum_ap):
            sl = slice(m * pw, (m + 1) * pw)
            nc.scalar.activation(dst[:, sl], psum_ap, ACT.Copy)
        return fn

    mm_pass(agA[:], ltpan, evict_plain(wpan))

    if debug_stage == "W":
        _dbg_finish(wpan)
        return

    # ---------- M = L W ----------
    mpan = pan_pool.tile([P, cw], F32, tag="pan")
    mm_pass(agLT[:], wpan, evict_plain(mpan))
    m_store = dram_pool.tile([P, cw], F32, tag="mstore")
    nc.sync.dma_start(m_store[:], mpan[:])

    if debug_stage == "M":
        _dbg_finish(mpan)
        return

    # persistent chain state
    t_cur = st_pool.tile([P, 1], F32)
    s_acc = st_pool.tile([P, 1], F32)
    ln_lam1 = st_pool.tile([P, 1], F32)
    mu = st_pool.tile([P, 1], F32)
    trMg = st_pool.tile([P, 1], F32)   # global trace of M

    def chain(x0_tile, K, init_t_fn, aux0=None):
        """Squaring chain; returns ln(lam_hat) as a [P,1] tile."""
        nc.vector.memset(s_acc[:], 0.0)
        xpan = x0_tile
        f_rep = fnorm_partial(xpan)
        for k in range(K + 1):
            tail = make_tail(f_rep, aux0 if k == 0 else None)
            if k < K:
                ag_out, totals = do_allgather(xpan, tail)
            else:
                totals = tiny_allgather(tail)
            if k == 0:
                init_t_fn(totals)
            tau = sm_pool.tile([P, 1], F32, tag="tau")
            nc.scalar.activation(tau[:], t_cur[:], ACT.Ln)
            if k < K:
                nc.vector.tensor_add(s_acc[:], s_acc[:], tau[:])
                nc.scalar.mul(s_acc[:], s_acc[:], 2.0)
            inv = sm_pool.tile([P, 1], F32, tag="inv")
            nc.vector.reciprocal(inv[:], t_cur[:])
            inv2 = sm_pool.tile([P, 1], F32, tag="inv2")
            nc.vector.tensor_tensor(out=inv2[:], in0=inv[:], in1=inv[:],
                                    op=ALU.mult)
            # t_next = F_tot / t^2
            nc.vector.tensor_tensor(out=t_cur[:], in0=totals[:, 0:1],
                                    in1=inv2[:], op=ALU.mult)
            if k == K:
                tau2 = sm_pool.tile([P, 1], F32, tag="tau2")
                nc.scalar.activation(tau2[:], t_cur[:], ACT.Ln)
                res = sm_pool.tile([P, 1], F32, tag="chainres")
                nc.vector.tensor_add(res[:], s_acc[:], tau[:])
                nc.vector.tensor_add(res[:], res[:], tau2[:])
                nc.scalar.mul(res[:], res[:], 1.0 / (2 ** K))
                return res
            xnew = pan_pool.tile([P, cw], F32, tag="pan")

            def evict_scaled(m, psum_ap, dst=xnew, sc=inv2):
                sl = slice(m * pw, (m + 1) * pw)
                nc.scalar.activation(dst[:, sl], psum_ap, ACT.Copy,
                                     scale=sc[:])
            mm_pass(ag_out[:], xpan, evict_scaled)
            xpan = xnew
            f_rep = fnorm_partial(xpan)
        raise AssertionError("unreachable")

    # ---------- chain 1: lambda_max of M ----------
    trM_loc = diag_partial(mpan)

    def init_t_chain1(totals):
        nc.vector.tensor_copy(t_cur[:], totals[:, 1:2])
        nc.vector.tensor_copy(trMg[:], totals[:, 1:2])

    if debug_stage == "T0":
        f_rep = fnorm_partial(mpan)
        tail = make_tail(f_rep, trM_loc)
        ag_out, totals = do_allgather(mpan, tail)
        init_t_chain1(totals)
        tau = sm_pool.tile([P, 1], F32, tag="tau")
        nc.scalar.activation(tau[:], t_cur[:], ACT.Ln)
        inv = sm_pool.tile([P, 1], F32, tag="inv")
        nc.vector.reciprocal(inv[:], t_cur[:])
        dbg = sm_pool.tile([1, 8], F32, tag="dbgv")
        nc.vector.memset(dbg[:], 0.0)
        nc.vector.tensor_copy(dbg[:, 0:1], t_cur[0:1, :])
        nc.vector.tensor_copy(dbg[:, 1:2], totals[0:1, 0:1])
        nc.vector.tensor_copy(dbg[:, 2:3], tau[0:1, :])
        nc.vector.tensor_copy(dbg[:, 3:4], inv[0:1, :])
        nc.vector.tensor_copy(dbg[:, 4:5], f_rep[0:1, :])
        nc.vector.tensor_copy(dbg[:, 5:6], trM_loc[0:1, :])
        nc.sync.dma_start(dbg_out[:], dbg[:])
        z = sm_pool.tile([1, 2], F32, tag="tail")
        nc.vector.memset(z[:], 0.0)
        nc.sync.dma_start(loss_out[:], z[0:1, 0:1])
        return

    res1 = chain(mpan, k1, init_t_chain1, aux0=trM_loc)
    nc.vector.tensor_copy(ln_lam1[:], res1[:])

    if debug_stage == "C1":
        dbg = sm_pool.tile([1, 8], F32, tag="dbgv")
        nc.vector.memset(dbg[:], 0.0)
        nc.vector.tensor_copy(dbg[:, 0:1], ln_lam1[0:1, :])
        nc.vector.tensor_copy(dbg[:, 1:2], trMg[0:1, :])
        nc.sync.dma_start(dbg_out[:], dbg[:])
        z = sm_pool.tile([1, 2], F32, tag="tail")
        nc.vector.memset(z[:], 0.0)
        nc.sync.dma_start(loss_out[:], z[0:1, 0:1])
        return

    # ---------- chain 2: mu - lambda_min via B = mu I - M ----------
    nc.scalar.activation(mu[:], ln_lam1[:], ACT.Exp)
    nc.scalar.mul(mu[:], mu[:], MU_FACTOR)

    mpan2 = pan_pool.tile([P, cw], F32, tag="pan")
    nc.sync.dma_start(mpan2[:], m_store[:])
    bpan = pan_pool.tile([P, cw], F32, tag="pan")
    nc.vector.tensor_scalar_mul(bpan[:], ei[:], mu[:])
    nc.vector.tensor_tensor(out=bpan[:], in0=bpan[:], in1=mpan2[:],
                            op=ALU.subtract)

    def init_t_chain2(totals):
        # t0 = n*mu - tr(M)
        nc.scalar.mul(t_cur[:], mu[:], float(n))
        nc.vector.tensor_tensor(out=t_cur[:], in0=t_cur[:],
                                in1=trMg[:], op=ALU.subtract)

    res2 = chain(bpan, k2, init_t_chain2)

    # ---------- final scalar math ----------
    bmax = sm_pool.tile([P, 1], F32, tag="bmax")
    nc.scalar.activation(bmax[:], res2[:], ACT.Exp)
    lam_min = sm_pool.tile([P, 1], F32, tag="lammin")
    nc.vector.tensor_tensor(out=lam_min[:], in0=mu[:], in1=bmax[:],
                            op=ALU.subtract)
    ln_min = sm_pool.tile([P, 1], F32, tag="lnmin")
    nc.scalar.activation(ln_min[:], lam_min[:], ACT.Ln)
    loss = sm_pool.tile([P, 1], F32, tag="lossv")
    nc.vector.tensor_tensor(out=loss[:], in0=ln_lam1[:], in1=ln_min[:],
                            op=ALU.subtract)
    nc.sync.dma_start(loss_out[:], loss[0:1, :])

    dbg = sm_pool.tile([1, 8], F32, tag="dbgv")
    nc.vector.tensor_copy(dbg[:, 0:1], ln_lam1[0:1, :])
    nc.vector.tensor_copy(dbg[:, 1:2], mu[0:1, :])
    nc.vector.tensor_copy(dbg[:, 2:3], bmax[0:1, :])
    nc.vector.tensor_copy(dbg[:, 3:4], lam_min[0:1, :])
    nc.vector.tensor_copy(dbg[:, 4:5], trMg[0:1, :])
    nc.vector.tensor_copy(dbg[:, 5:6], loss[0:1, :])
    nc.sync.dma_start(dbg_out[:], dbg[:])


_NC_CACHE = {}


def _get_nc(n=2048, k1=K1, k2=K2, mm_dt=F32):
    key = (n, k1, k2, str(mm_dt))
    if key not in _NC_CACHE:
        _NC_CACHE[key] = _build_nc(n, k1, k2, mm_dt)
    return _NC_CACHE[key]


def _panelize(mat, i, n):
    """[128, (n//128)*(n//8)] panel of mat[:, i*pw:(i+1)*pw] in SBUF chunk
    layout pan[p, c*pw+j] = mat[c*128+p, i*pw+j]."""
    pw = n // N_CORES
    ch = n // P
    x = mat[:, i * pw:(i + 1) * pw].reshape(ch, P, pw)
    return np.ascontiguousarray(x.transpose(1, 0, 2).reshape(P, ch * pw))


def _prep_inputs(pred_values, active_scales, A_factor, factor_rows,
                 factor_cols, n):
    G = np.asarray(A_factor, dtype=np.float32)
    GT = np.ascontiguousarray(G.T)
    vals = (np.asarray(pred_values, dtype=np.float32)
            * np.asarray(active_scales, dtype=np.float32))
    LT = np.eye(n, dtype=np.float32)
    np.add.at(LT, (np.asarray(factor_cols), np.asarray(factor_rows)), vals)
    eye = np.eye(n, dtype=np.float32)
    in_maps = []
    for i in range(N_CORES):
        in_maps.append({
            "git_pan": _panelize(GT, i, n),
            "lti_pan": _panelize(LT, i, n),
            "ei_pan": _panelize(eye, i, n),
        })
    return in_maps


_RUNNER_CACHE = {}


def _make_pjrt_runner(nc):
    """Cached jit(shard_map) runner for the axon/PJRT path: avoids the
    per-call retrace that run_bass_via_pjrt pays, so repeat kernel() calls
    cost transfer + execute only."""
    import jax
    from jax.sharding import Mesh, PartitionSpec
    try:
        from jax.experimental.shard_map import shard_map
    except Exception:
        from jax.shard_map import shard_map  # newer jax
    from concourse import bass2jax
    from concourse import mybir as _mybir

    bass2jax.install_neuronx_cc_hook()
    partition_name = (nc.partition_id_tensor.name
                      if nc.partition_id_tensor else None)
    in_names, out_names, out_avals, zero_shapes = [], [], [], []
    for alloc in nc.m.functions[0].allocations:
        if not isinstance(alloc, _mybir.MemoryLocationSet):
            continue
        name = alloc.memorylocations[0].name
        if alloc.kind == "ExternalInput":
            if name != partition_name:
                in_names.append(name)
        elif alloc.kind == "ExternalOutput":
            out_names.append(name)
            shape = tuple(alloc.tensor_shape)
            dtype = _mybir.dt.np(alloc.dtype)
            out_avals.append(jax.core.ShapedArray(shape, dtype))
            zero_shapes.append((shape, dtype))
    n_params = len(in_names)
    all_in_names = list(in_names) + list(out_names)
    if partition_name is not None:
        all_in_names.append(partition_name)
    donate = tuple(range(n_params, n_params + len(out_names)))

    def _body(*args):
        operands = list(args)
        if partition_name is not None:
            operands.append(bass2jax.partition_id_tensor())
        outs = bass2jax._bass_exec_p.bind(
            *operands,
            out_avals=tuple(out_avals),
            in_names=tuple(all_in_names),
            out_names=tuple(out_names),
            lowering_input_output_aliases=(),
            sim_require_finite=True,
            sim_require_nnan=True,
            nc=nc,
        )
        return tuple(outs)

    devices = jax.devices()[:N_CORES]
    mesh = Mesh(np.asarray(devices), ("core",))
    n_args = n_params + len(out_names)
    sharded = jax.jit(
        shard_map(_body, mesh=mesh,
                  in_specs=(PartitionSpec("core"),) * n_args,
                  out_specs=(PartitionSpec("core"),) * len(out_names),
                  check_rep=False),
        donate_argnums=donate, keep_unused=True)

    def run(in_maps):
        concat_in = [
            np.concatenate([np.asarray(in_maps[c][nm]) for c in range(N_CORES)],
                           axis=0)
            for nm in in_names
        ]
        concat_zeros = [
            np.zeros((N_CORES * s[0],) + tuple(s[1:]), dt)
            for (s, dt) in zero_shapes
        ]
        out_arrs = sharded(*concat_in, *concat_zeros)
        res = []
        for c in range(N_CORES):
            res.append({
                nm: np.asarray(out_arrs[i]).reshape(
                    N_CORES, *out_avals[i].shape)[c]
                for i, nm in enumerate(out_names)
            })
        return res

    return run


def _run(nc, in_maps):
    from concourse._compat import axon_active
    if axon_active():
        key = id(nc)
        if key not in _RUNNER_CACHE:
            _RUNNER_CACHE[key] = _make_pjrt_runner(nc)
        return _RUNNER_CACHE[key](in_maps)
    return run_bass_kernel_spmd(
        nc, in_maps, core_ids=list(range(N_CORES))).results


def kernel(pred_values, active_scales, A_factor, factor_rows, factor_cols):
    n = A_factor.shape[0]
    nc = _get_nc(n=n)
    in_maps = _prep_inputs(pred_values, active_scales, A_factor,
                           factor_rows, factor_cols, n)
    results = _run(nc, in_maps)
    out = results[0]["loss"]
    return np.float32(out[0, 0])


if __name__ == "__main__":
    import reference, jax
    cpu = jax.devices("cpu")[0]
    with jax.default_device(cpu):
        inputs = {k: np.asarray(v) for k, v in reference.setup_inputs().items()}
    got = kernel(**inputs)
    print("kernel loss:", got)


# revision 33
# speedup vs baseline: 724.4065x; 1.6438x over previous
"""Trainium2 Bass kernel for nn_CachedConditionNumberLoss.

Computes loss = log(lambda_max) - log(lambda_min) of M = L A L^T where
A = G G^T/n + I  (G = A_factor, n = 2048) and L = I + scatter(pred*scale).

Strategy (8-core SPMD, column-panel sharded):
  - core i owns the column panel X[:, i*PW:(i+1)*PW] of every 2048x2048
    matrix involved; all cross-core exchange is AllGather of panels.
  - device computes A = G G^T/n + I, W = A L^T, M = L W (three panel
    matmul passes), then extremal eigenvalues of M via two repeated-
    squaring chains with trace-ratio estimators:
      chain 1 on M          -> lambda_max
      chain 2 on mu*I - M   -> mu - lambda_min   (mu = 1.001*lambda_max)
    Each squaring step: AllGather panels of X_k (with the Frobenius-norm
    partial embedded in a tail row), then each core computes
    X_{k+1}[:, panel] = (X_k^T X_k)[:, panel] / t_k^2 with PE matmuls
    (lhsT tiles are read straight from the gathered copy; symmetry of X_k
    makes transposes unnecessary).  Trace bookkeeping on device:
      t_{k+1} = ||X_k||_F^2 / t_k^2,  tau_k = ln t_k,
      s_{k+1} = 2 (s_k + tau_k),
      ln(lam_hat) = (s_K + tau_K + tau_{K+1}) / 2^K.
  - host only transposes/slices inputs (incl. assembling L^T from the
    scatter triplets) and reads back the scalar.
"""

import numpy as np

import concourse.tile as tile
from concourse import bacc, mybir
from concourse.bass_utils import run_bass_kernel_spmd

F32 = mybir.dt.float32
ACT = mybir.ActivationFunctionType
ALU = mybir.AluOpType
P = 128
N_CORES = 8

# squaring-chain lengths (validated in fp32 simulation: loss relerr ~2e-7)
K1 = 10
K2 = 14
MU_FACTOR = 1.001


def _build_nc(n=2048, k1=K1, k2=K2, mm_dt=F32, debug_stage=None, repeats=1):
    ch = n // P           # 128-row chunks per matrix (16)
    pw = n // N_CORES     # panel width per core (256)
    cw = ch * pw          # panel free size in SBUF layout (4096)
    agr = P + 1           # rows per rank in AG buffers (tail row at P)
    cpp = pw // P         # column chunks per panel (2)

    nc = bacc.Bacc(None, target_bir_lowering=False)

    git_pan = nc.dram_tensor("git_pan", [P, cw], F32, kind="ExternalInput")
    lti_pan = nc.dram_tensor("lti_pan", [P, cw], F32, kind="ExternalInput")
    ei_pan = nc.dram_tensor("ei_pan", [P, cw], F32, kind="ExternalInput")

    loss_out = nc.dram_tensor("loss", [1, 1], F32, kind="ExternalOutput")
    dbg_out = nc.dram_tensor("dbg", [1, 8], F32, kind="ExternalOutput")

    pan_out = (nc.dram_tensor("pan_out", [P, cw], F32, kind="ExternalOutput")
               if debug_stage in ("A", "W", "M") else None)

    with tile.TileContext(nc) as tc:
        with (
            tc.tile_pool(name="xf", bufs=7) as xf_pool,
            tc.tile_pool(name="pan", bufs=3) as pan_pool,
            tc.tile_pool(name="eip", bufs=1) as ei_pool,
            tc.tile_pool(name="small", bufs=6) as sm_pool,
            tc.tile_pool(name="state", bufs=1) as st_pool,
            tc.tile_pool(name="psum", bufs=6, space="PSUM") as ps_pool,
            tc.tile_pool(name="psr", bufs=2, space="PSUM") as psr_pool,
            tc.tile_pool(name="dram", bufs=2, space="DRAM") as dram_pool,
        ):
            for _rep in range(repeats):
                _trace_program(
                    nc, n, k1, k2, mm_dt, debug_stage,
                    ch, pw, cw, agr, cpp,
                    git_pan, lti_pan, ei_pan,
                    loss_out, dbg_out, pan_out,
                    xf_pool, pan_pool, ei_pool, sm_pool, st_pool,
                    ps_pool, psr_pool, dram_pool,
                )

    nc.compile()
    return nc


def _trace_program(nc, n, k1, k2, mm_dt, debug_stage,
                   ch, pw, cw, agr, cpp,
                   git_pan, lti_pan, ei_pan,
                   loss_out, dbg_out, pan_out,
                   xf_pool, pan_pool, ei_pool, sm_pool, st_pool,
                   ps_pool, psr_pool, dram_pool):
    ones = st_pool.tile([P, P], F32)
    nc.vector.memset(ones[:], 1.0)

    ei = ei_pool.tile([P, cw], F32, tag="ei")
    nc.sync.dma_start(ei[:], ei_pan[:])

    # ---------- helpers ----------
    def part_reduce(vec_ap, width=1):
        """[p, width] -> [P, width] replicated column sums."""
        red = psr_pool.tile([P, 2], F32, space="PSUM", tag="red")
        p_sz = vec_ap.shape[0]
        nc.tensor.matmul(red[:, 0:width], lhsT=ones[:p_sz, :],
                         rhs=vec_ap, start=True, stop=True)
        out = sm_pool.tile([P, width], F32, tag="pred")
        nc.vector.tensor_copy(out[:], red[:, 0:width])
        return out

    def fnorm_partial(pan_tile):
        """sum of squares of a [P, cw] panel -> [P,1] replicated."""
        acc = sm_pool.tile([P, ch], F32, tag="facc")
        for c in range(ch):
            tmp = sm_pool.tile([P, pw], F32, tag="sqtmp")
            nc.scalar.activation(tmp[:], pan_tile[:, c * pw:(c + 1) * pw],
                                 ACT.Square, accum_out=acc[:, c:c + 1])
        accs = sm_pool.tile([P, 1], F32, tag="faccs")
        nc.vector.reduce_sum(accs[:], acc[:], axis=mybir.AxisListType.X)
        return part_reduce(accs[:])

    def diag_partial(pan_tile):
        """sum of (panel .* ei) -> [P,1] replicated local trace partial."""
        acc = sm_pool.tile([P, ch], F32, tag="facc")
        for c in range(ch):
            sl = slice(c * pw, (c + 1) * pw)
            tmp = sm_pool.tile([P, pw], F32, tag="sqtmp")
            nc.vector.tensor_tensor(out=tmp[:], in0=pan_tile[:, sl],
                                    in1=ei[:, sl], op=ALU.mult)
            nc.vector.reduce_sum(acc[:, c:c + 1], tmp[:],
                                 axis=mybir.AxisListType.X)
        accs = sm_pool.tile([P, 1], F32, tag="faccs")
        nc.vector.reduce_sum(accs[:], acc[:], axis=mybir.AxisListType.X)
        return part_reduce(accs[:])

    f32r = (mm_dt == mybir.dt.float32r)

    def _mmcast(ap):
        return ap.bitcast(mybir.dt.float32r) if f32r else ap

    def make_fused_evict(dst, scale_ap=None, prefill=True, with_diag=False,
                         scale_const=None, add_ei=False, with_fnorm=True):
        """Eviction callback computing chunk square-sums (DVE, overlaps the
        ACT copy) and streaming finished chunks into the next AG input."""
        facc = sm_pool.tile([P, ch], F32, tag="facc")
        dacc = (sm_pool.tile([P, ch], F32, tag="dacc", name="dacc")
                if with_diag else None)
        ag_in_next = (dram_pool.tile([agr, cw], F32, tag="agin",
                                     name="ag_in_next")
                      if prefill else None)

        def evict(m, psum_ap):
            sl = slice(m * pw, (m + 1) * pw)
            if scale_ap is not None:
                nc.scalar.activation(dst[:, sl], psum_ap, ACT.Copy,
                                     scale=scale_ap[:])
            elif scale_const is not None:
                nc.scalar.activation(dst[:, sl], psum_ap, ACT.Copy,
                                     scale=scale_const)
            else:
                nc.scalar.activation(dst[:, sl], psum_ap, ACT.Copy)
            if add_ei:
                nc.vector.tensor_add(dst[:, sl], dst[:, sl], ei[:, sl])
            if with_fnorm:
                # square-sum of the raw psum chunk (scaling folded in later)
                tmp = sm_pool.tile([P, pw], F32, tag="sqtmp")
                nc.scalar.activation(tmp[:], psum_ap, ACT.Square,
                                     accum_out=facc[:, m:m + 1])
            if with_diag:
                tmp2 = sm_pool.tile([P, pw], F32, tag="sqtmp")
                nc.vector.tensor_tensor(out=tmp2[:], in0=dst[:, sl],
                                        in1=ei[:, sl], op=ALU.mult)
                nc.vector.reduce_sum(dacc[:, m:m + 1], tmp2[:],
                                     axis=mybir.AxisListType.X)
            if ag_in_next is not None:
                nc.sync.dma_start(ag_in_next[0:P, sl], dst[:, sl])

        return evict, facc, dacc, ag_in_next

    def finish_fnorm(facc, scale2_ap=None):
        """facc [P,ch] chunk sums -> replicated total, x scale_ap^2."""
        accs = sm_pool.tile([P, 1], F32, tag="faccs")
        nc.vector.reduce_sum(accs[:], facc[:], axis=mybir.AxisListType.X)
        if scale2_ap is not None:
            nc.vector.tensor_tensor(out=accs[:], in0=accs[:],
                                    in1=scale2_ap[:], op=ALU.mult)
            nc.vector.tensor_tensor(out=accs[:], in0=accs[:],
                                    in1=scale2_ap[:], op=ALU.mult)
        return part_reduce(accs[:])

    def mm_pass(src_dram, rhs_tile, evict_fn):
        """out[:, panel] = X^T @ rhs_panel, X stored panelized in src_dram."""
        tiles = []
        for r in range(N_CORES):
            t = xf_pool.tile([P, cw], F32, tag="xf")
            nc.sync.dma_start(t[:], src_dram[r * agr:r * agr + P, :])
            tiles.append(t)
        for m in range(ch):
            acc = ps_pool.tile([P, pw], F32, space="PSUM", tag="mm")
            t = tiles[m // cpp]
            base = (m % cpp) * P
            for k in range(ch):
                nc.tensor.matmul(
                    acc[:],
                    lhsT=_mmcast(t[:, k * pw + base:k * pw + base + P]),
                    rhs=_mmcast(rhs_tile[:, k * pw:(k + 1) * pw]),
                    start=(k == 0), stop=(k == ch - 1),
                )
            evict_fn(m, acc[:])

    def do_allgather(pan_tile, tail_tile, pre_ag_in=None):
        """AG panels + 2-value tail; returns (ag_out, totals[P,2]).
        pre_ag_in: ag_in tile already filled chunk-wise during the
        producing matmul pass (skips the bulk 2MB copy here)."""
        if pre_ag_in is None:
            ag_in = dram_pool.tile([agr, cw], F32, tag="agin")
            nc.sync.dma_start(ag_in[0:P, :], pan_tile[:])
        else:
            ag_in = pre_ag_in
        ag_out = dram_pool.tile([N_CORES * agr, cw], F32, tag="agout",
                                addr_space="Shared")
        nc.sync.dma_start(ag_in[P:P + 1, 0:2], tail_tile[0:1, 0:2])
        nc.gpsimd.collective_compute(
            "AllGather", ALU.bypass,
            ins=[ag_in[:]], outs=[ag_out[:]],
            replica_groups=[list(range(N_CORES))],
        )
        tails8 = sm_pool.tile([N_CORES, 2], F32, tag="tails8")
        nc.sync.dma_start(
            tails8[:],
            ag_out.rearrange("(r p) c -> r p c", p=agr)[:, P:P + 1, 0:2])
        totals = part_reduce(tails8[:], width=2)
        return ag_out, totals

    def tiny_allgather(tail_tile):
        agt_in = dram_pool.tile([1, 16], F32, tag="agtin")
        agt_out = dram_pool.tile([N_CORES, 16], F32, tag="agtout",
                                 addr_space="Shared")
        pad = sm_pool.tile([1, 16], F32, tag="tailpad")
        nc.vector.memset(pad[:], 0.0)
        nc.vector.tensor_copy(pad[:, 0:2], tail_tile[0:1, 0:2])
        nc.sync.dma_start(agt_in[:], pad[:])
        nc.gpsimd.collective_compute(
            "AllGather", ALU.bypass,
            ins=[agt_in[:]], outs=[agt_out[:]],
            replica_groups=[list(range(N_CORES))],
        )
        t8 = sm_pool.tile([N_CORES, 2], F32, tag="tails8")
        nc.sync.dma_start(t8[:], agt_out[:, 0:2])
        return part_reduce(t8[:], width=2)

    def make_tail(f_rep, aux_rep=None):
        t = sm_pool.tile([1, 2], F32, tag="tail")
        nc.vector.tensor_copy(t[:, 0:1], f_rep[0:1, :])
        if aux_rep is not None:
            nc.vector.tensor_copy(t[:, 1:2], aux_rep[0:1, :])
        else:
            nc.vector.memset(t[:, 1:2], 0.0)
        return t

    def _dbg_finish(tile_):
        nc.sync.dma_start(pan_out[:], tile_[:])
        z = sm_pool.tile([1, 2], F32, tag="tail")
        nc.vector.memset(z[:], 0.0)
        nc.sync.dma_start(loss_out[:], z[0:1, 0:1])
        d = sm_pool.tile([1, 8], F32, tag="dbgv")
        nc.vector.memset(d[:], 0.0)
        nc.sync.dma_start(dbg_out[:], d[:])

    # ---------- formation: A = G G^T / n + I ----------
    gpan = pan_pool.tile([P, cw], F32, tag="pan")
    nc.sync.dma_start(gpan[:], git_pan[:])

    zt0 = sm_pool.tile([1, 2], F32, tag="tail")
    nc.vector.memset(zt0[:], 0.0)
    agG, _ = do_allgather(gpan, zt0)         # full G^T, panelized

    # L^T panels: AG early too (full L^T needed for the M pass)
    ltpan = pan_pool.tile([P, cw], F32, tag="pan")
    nc.sync.dma_start(ltpan[:], lti_pan[:])
    zt1 = sm_pool.tile([1, 2], F32, tag="tail")
    nc.vector.memset(zt1[:], 0.0)
    agLT, _ = do_allgather(ltpan, zt1)       # full L^T, panelized

    apan = pan_pool.tile([P, cw], F32, tag="pan")
    evict_a, _, _, agA_in = make_fused_evict(apan, scale_const=1.0 / n,
                                             prefill=True, add_ei=True,
                                             with_fnorm=False)
    mm_pass(agG[:], gpan, evict_a)

    if debug_stage == "A":
        _dbg_finish(apan)
        return

    # ---------- AG(A); W = A L^T ----------
    zt = sm_pool.tile([1, 2], F32, tag="tail")
    nc.vector.memset(zt[:], 0.0)
    agA, _ = do_allgather(apan, zt, pre_ag_in=agA_in)

    wpan = pan_pool.tile([P, cw], F32, tag="pan")

    def evict_plain(dst):
        def fn(m, psum_ap):
            sl = slice(m * pw, (m + 1) * pw)
            nc.scalar.activation(dst[:, sl], psum_ap, ACT.Copy)
        return fn

    mm_pass(agA[:], ltpan, evict_plain(wpan))

    if debug_stage == "W":
        _dbg_finish(wpan)
        return

    # ---------- M = L W ----------
    mpan = pan_pool.tile([P, cw], F32, tag="pan")
    evict_m, m_facc, m_dacc, m_agin = make_fused_evict(
        mpan, prefill=True, with_diag=True)
    mm_pass(agLT[:], wpan, evict_m)
    m_store = dram_pool.tile([P, cw], F32, tag="mstore")
    nc.sync.dma_start(m_store[:], mpan[:])

    if debug_stage == "M":
        _dbg_finish(mpan)
        return

    # persistent chain state
    t_cur = st_pool.tile([P, 1], F32)
    s_acc = st_pool.tile([P, 1], F32)
    ln_lam1 = st_pool.tile([P, 1], F32)
    mu = st_pool.tile([P, 1], F32)
    trMg = st_pool.tile([P, 1], F32)   # global trace of M

    def chain(x0_tile, K, init_t_fn, aux0=None, pre0=None):
        """Squaring chain; returns ln(lam_hat) as a [P,1] tile.
        pre0: optional (f_rep, ag_in) produced by the pass that formed x0
        (fnorm and AG-input streaming fused into its evictions)."""
        nc.vector.memset(s_acc[:], 0.0)
        xpan = x0_tile
        if pre0 is None:
            f_rep = fnorm_partial(xpan)
            ag_in_pre = None
        else:
            f_rep, ag_in_pre = pre0
        for k in range(K + 1):
            tail = make_tail(f_rep, aux0 if k == 0 else None)
            if k < K:
                ag_out, totals = do_allgather(xpan, tail,
                                              pre_ag_in=ag_in_pre)
            else:
                totals = tiny_allgather(tail)
            if k == 0:
                init_t_fn(totals)
            tau = sm_pool.tile([P, 1], F32, tag="tau")
            nc.scalar.activation(tau[:], t_cur[:], ACT.Ln)
            if k < K:
                nc.vector.tensor_add(s_acc[:], s_acc[:], tau[:])
                nc.scalar.mul(s_acc[:], s_acc[:], 2.0)
            inv = sm_pool.tile([P, 1], F32, tag="inv")
            nc.vector.reciprocal(inv[:], t_cur[:])
            inv2 = sm_pool.tile([P, 1], F32, tag="inv2")
            nc.vector.tensor_tensor(out=inv2[:], in0=inv[:], in1=inv[:],
                                    op=ALU.mult)
            # t_next = F_tot / t^2
            nc.vector.tensor_tensor(out=t_cur[:], in0=totals[:, 0:1],
                                    in1=inv2[:], op=ALU.mult)
            if k == K:
                tau2 = sm_pool.tile([P, 1], F32, tag="tau2")
                nc.scalar.activation(tau2[:], t_cur[:], ACT.Ln)
                res = sm_pool.tile([P, 1], F32, tag="chainres")
                nc.vector.tensor_add(res[:], s_acc[:], tau[:])
                nc.vector.tensor_add(res[:], res[:], tau2[:])
                nc.scalar.mul(res[:], res[:], 1.0 / (2 ** K))
                return res
            xnew = pan_pool.tile([P, cw], F32, tag="pan")
            evict_scaled, facc, _, ag_in_pre = make_fused_evict(
                xnew, scale_ap=inv2, prefill=(k + 1 < K))
            mm_pass(ag_out[:], xpan, evict_scaled)
            xpan = xnew
            # ||xnew||_F^2 = inv2^2 * sum(psum^2)
            f_rep = finish_fnorm(facc, scale2_ap=inv2)
        raise AssertionError("unreachable")

    # ---------- chain 1: lambda_max of M ----------
    trM_loc = finish_fnorm(m_dacc)      # sum of diag partials (no scaling)
    m_frep = finish_fnorm(m_facc)

    def init_t_chain1(totals):
        nc.vector.tensor_copy(t_cur[:], totals[:, 1:2])
        nc.vector.tensor_copy(trMg[:], totals[:, 1:2])

    if debug_stage == "T0":
        f_rep = fnorm_partial(mpan)
        tail = make_tail(f_rep, trM_loc)
        ag_out, totals = do_allgather(mpan, tail)
        init_t_chain1(totals)
        tau = sm_pool.tile([P, 1], F32, tag="tau")
        nc.scalar.activation(tau[:], t_cur[:], ACT.Ln)
        inv = sm_pool.tile([P, 1], F32, tag="inv")
        nc.vector.reciprocal(inv[:], t_cur[:])
        dbg = sm_pool.tile([1, 8], F32, tag="dbgv")
        nc.vector.memset(dbg[:], 0.0)
        nc.vector.tensor_copy(dbg[:, 0:1], t_cur[0:1, :])
        nc.vector.tensor_copy(dbg[:, 1:2], totals[0:1, 0:1])
        nc.vector.tensor_copy(dbg[:, 2:3], tau[0:1, :])
        nc.vector.tensor_copy(dbg[:, 3:4], inv[0:1, :])
        nc.vector.tensor_copy(dbg[:, 4:5], f_rep[0:1, :])
        nc.vector.tensor_copy(dbg[:, 5:6], trM_loc[0:1, :])
        nc.sync.dma_start(dbg_out[:], dbg[:])
        z = sm_pool.tile([1, 2], F32, tag="tail")
        nc.vector.memset(z[:], 0.0)
        nc.sync.dma_start(loss_out[:], z[0:1, 0:1])
        return

    res1 = chain(mpan, k1, init_t_chain1, aux0=trM_loc,
                 pre0=(m_frep, m_agin))
    nc.vector.tensor_copy(ln_lam1[:], res1[:])

    if debug_stage == "C1":
        dbg = sm_pool.tile([1, 8], F32, tag="dbgv")
        nc.vector.memset(dbg[:], 0.0)
        nc.vector.tensor_copy(dbg[:, 0:1], ln_lam1[0:1, :])
        nc.vector.tensor_copy(dbg[:, 1:2], trMg[0:1, :])
        nc.sync.dma_start(dbg_out[:], dbg[:])
        z = sm_pool.tile([1, 2], F32, tag="tail")
        nc.vector.memset(z[:], 0.0)
        nc.sync.dma_start(loss_out[:], z[0:1, 0:1])
        return

    # ---------- chain 2: mu - lambda_min via B = mu I - M ----------
    nc.scalar.activation(mu[:], ln_lam1[:], ACT.Exp)
    nc.scalar.mul(mu[:], mu[:], MU_FACTOR)

    mpan2 = pan_pool.tile([P, cw], F32, tag="pan")
    nc.sync.dma_start(mpan2[:], m_store[:])
    bpan = pan_pool.tile([P, cw], F32, tag="pan")
    nc.vector.tensor_scalar_mul(bpan[:], ei[:], mu[:])
    nc.vector.tensor_tensor(out=bpan[:], in0=bpan[:], in1=mpan2[:],
                            op=ALU.subtract)

    def init_t_chain2(totals):
        # t0 = n*mu - tr(M)
        nc.scalar.mul(t_cur[:], mu[:], float(n))
        nc.vector.tensor_tensor(out=t_cur[:], in0=t_cur[:],
                                in1=trMg[:], op=ALU.subtract)

    res2 = chain(bpan, k2, init_t_chain2)

    # ---------- final scalar math ----------
    bmax = sm_pool.tile([P, 1], F32, tag="bmax")
    nc.scalar.activation(bmax[:], res2[:], ACT.Exp)
    lam_min = sm_pool.tile([P, 1], F32, tag="lammin")
    nc.vector.tensor_tensor(out=lam_min[:], in0=mu[:], in1=bmax[:],
                            op=ALU.subtract)
    ln_min = sm_pool.tile([P, 1], F32, tag="lnmin")
    nc.scalar.activation(ln_min[:], lam_min[:], ACT.Ln)
    loss = sm_pool.tile([P, 1], F32, tag="lossv")
    nc.vector.tensor_tensor(out=loss[:], in0=ln_lam1[:], in1=ln_min[:],
                            op=ALU.subtract)
    nc.sync.dma_start(loss_out[:], loss[0:1, :])

    dbg = sm_pool.tile([1, 8], F32, tag="dbgv")
    nc.vector.tensor_copy(dbg[:, 0:1], ln_lam1[0:1, :])
    nc.vector.tensor_copy(dbg[:, 1:2], mu[0:1, :])
    nc.vector.tensor_copy(dbg[:, 2:3], bmax[0:1, :])
    nc.vector.tensor_copy(dbg[:, 3:4], lam_min[0:1, :])
    nc.vector.tensor_copy(dbg[:, 4:5], trMg[0:1, :])
    nc.vector.tensor_copy(dbg[:, 5:6], loss[0:1, :])
    nc.sync.dma_start(dbg_out[:], dbg[:])


_NC_CACHE = {}


def _get_nc(n=2048, k1=K1, k2=K2, mm_dt=F32):
    key = (n, k1, k2, str(mm_dt))
    if key not in _NC_CACHE:
        _NC_CACHE[key] = _build_nc(n, k1, k2, mm_dt)
    return _NC_CACHE[key]


def _panelize(mat, i, n):
    """[128, (n//128)*(n//8)] panel of mat[:, i*pw:(i+1)*pw] in SBUF chunk
    layout pan[p, c*pw+j] = mat[c*128+p, i*pw+j]."""
    pw = n // N_CORES
    ch = n // P
    x = mat[:, i * pw:(i + 1) * pw].reshape(ch, P, pw)
    return np.ascontiguousarray(x.transpose(1, 0, 2).reshape(P, ch * pw))


def _prep_inputs(pred_values, active_scales, A_factor, factor_rows,
                 factor_cols, n):
    G = np.asarray(A_factor, dtype=np.float32)
    GT = np.ascontiguousarray(G.T)
    vals = (np.asarray(pred_values, dtype=np.float32)
            * np.asarray(active_scales, dtype=np.float32))
    LT = np.eye(n, dtype=np.float32)
    np.add.at(LT, (np.asarray(factor_cols), np.asarray(factor_rows)), vals)
    eye = np.eye(n, dtype=np.float32)
    in_maps = []
    for i in range(N_CORES):
        in_maps.append({
            "git_pan": _panelize(GT, i, n),
            "lti_pan": _panelize(LT, i, n),
            "ei_pan": _panelize(eye, i, n),
        })
    return in_maps


_RUNNER_CACHE = {}


def _make_pjrt_runner(nc):
    """Cached jit(shard_map) runner for the axon/PJRT path: avoids the
    per-call retrace that run_bass_via_pjrt pays, so repeat kernel() calls
    cost transfer + execute only."""
    import jax
    from jax.sharding import Mesh, PartitionSpec
    try:
        from jax.experimental.shard_map import shard_map
    except Exception:
        from jax.shard_map import shard_map  # newer jax
    from concourse import bass2jax
    from concourse import mybir as _mybir

    bass2jax.install_neuronx_cc_hook()
    partition_name = (nc.partition_id_tensor.name
                      if nc.partition_id_tensor else None)
    in_names, out_names, out_avals, zero_shapes = [], [], [], []
    for alloc in nc.m.functions[0].allocations:
        if not isinstance(alloc, _mybir.MemoryLocationSet):
            continue
        name = alloc.memorylocations[0].name
        if alloc.kind == "ExternalInput":
            if name != partition_name:
                in_names.append(name)
        elif alloc.kind == "ExternalOutput":
            out_names.append(name)
            shape = tuple(alloc.tensor_shape)
            dtype = _mybir.dt.np(alloc.dtype)
            out_avals.append(jax.core.ShapedArray(shape, dtype))
            zero_shapes.append((shape, dtype))
    n_params = len(in_names)
    all_in_names = list(in_names) + list(out_names)
    if partition_name is not None:
        all_in_names.append(partition_name)
    donate = tuple(range(n_params, n_params + len(out_names)))

    def _body(*args):
        operands = list(args)
        if partition_name is not None:
            operands.append(bass2jax.partition_id_tensor())
        outs = bass2jax._bass_exec_p.bind(
            *operands,
            out_avals=tuple(out_avals),
            in_names=tuple(all_in_names),
            out_names=tuple(out_names),
            lowering_input_output_aliases=(),
            sim_require_finite=True,
            sim_require_nnan=True,
            nc=nc,
        )
        return tuple(outs)

    devices = jax.devices()[:N_CORES]
    mesh = Mesh(np.asarray(devices), ("core",))
    n_args = n_params + len(out_names)
    sharded = jax.jit(
        shard_map(_body, mesh=mesh,
                  in_specs=(PartitionSpec("core"),) * n_args,
                  out_specs=(PartitionSpec("core"),) * len(out_names),
                  check_rep=False),
        donate_argnums=donate, keep_unused=True)

    def run(in_maps):
        concat_in = [
            np.concatenate([np.asarray(in_maps[c][nm]) for c in range(N_CORES)],
                           axis=0)
            for nm in in_names
        ]
        concat_zeros = [
            np.zeros((N_CORES * s[0],) + tuple(s[1:]), dt)
            for (s, dt) in zero_shapes
        ]
        out_arrs = sharded(*concat_in, *concat_zeros)
        res = []
        for c in range(N_CORES):
            res.append({
                nm: np.asarray(out_arrs[i]).reshape(
                    N_CORES, *out_avals[i].shape)[c]
                for i, nm in enumerate(out_names)
            })
        return res

    return run


def _run(nc, in_maps):
    from concourse._compat import axon_active
    if axon_active():
        key = id(nc)
        if key not in _RUNNER_CACHE:
            _RUNNER_CACHE[key] = _make_pjrt_runner(nc)
        return _RUNNER_CACHE[key](in_maps)
    return run_bass_kernel_spmd(
        nc, in_maps, core_ids=list(range(N_CORES))).results


def kernel(pred_values, active_scales, A_factor, factor_rows, factor_cols):
    n = A_factor.shape[0]
    nc = _get_nc(n=n)
    in_maps = _prep_inputs(pred_values, active_scales, A_factor,
                           factor_rows, factor_cols, n)
    results = _run(nc, in_maps)
    out = results[0]["loss"]
    return np.float32(out[0, 0])


if __name__ == "__main__":
    import reference, jax
    cpu = jax.devices("cpu")[0]
    with jax.default_device(cpu):
        inputs = {k: np.asarray(v) for k, v in reference.setup_inputs().items()}
    got = kernel(**inputs)
    print("kernel loss:", got)
